# revision 2
# baseline (speedup 1.0000x reference)
"""Trainium2 Bass kernel v2 for nn_CALayer — halves-stacked layout.

Data parallel over batch B=8 across 8 cores; within a core the image's
two row-halves (rows 0-63 / 64-127) are stacked on SBUF partitions
0-63 / 64-127.  All 64-channel convs become 128-wide block-diagonal
matmuls (half the PE passes of v1), and all element-wise work runs at
full 128-lane DVE width.  The per-pixel kernel broadcast (group row ->
8 channels) runs on the DMA engines via stride-0 access patterns
instead of PE matmuls.
"""

import numpy as np

import concourse.bass as bass
import concourse.tile as tile
from concourse import mybir
from concourse.bass_utils import run_bass_kernel_spmd

F32 = mybir.dt.float32
BF16 = mybir.dt.bfloat16

H = 128
W = 128
C = 64
HH = 64          # rows per half
NPIX = H * W
NPIXH = HH * W   # 8192 pixels per half
PH2 = 66         # padded rows per half (1 + 64 + 1)
PW = 132         # padded width
CH = 512         # psum chunk columns (4 rows per half)
NCH = NPIXH // CH    # 16 chunks
RPC = CH // W        # 4 rows per chunk
BQ = 2048        # big chunk for stage C / loads (16 rows per half)
NBQ = NPIXH // BQ    # 4
RBQ = BQ // W        # 16
EPS = 1e-5

AF = mybir.ActivationFunctionType
ALU = mybir.AluOpType


def _split_big_waits(nc, max_waits=1):
    """walrus CTRL codegen accepts only one sem wait per instruction; move
    extra waits onto Drain instructions inserted just before."""
    from concourse import mybir as _mybir
    n_fixed = 0
    for fn in nc.m.functions:
        for bb in fn.blocks:
            insts = bb.instructions
            i = 0
            while i < len(insts):
                inst = insts[i]
                si = inst.sync_info
                if si is not None and si.on_wait and len(si.on_wait) > max_waits:
                    waits = list(si.on_wait)
                    keep = waits[-max_waits:]
                    extra = waits[:-max_waits]
                    new_insts = []
                    for j in range(0, len(extra), max_waits):
                        chunk = extra[j : j + max_waits]
                        d = _mybir.InstDrain(
                            name=f"{inst.name}-waitsplit{j}", ins=[], outs=[]
                        )
                        d.engine = inst.engine
                        d.sync_info = _mybir.SyncInfo(on_wait=chunk, on_update=[])
                        new_insts.append(d)
                    si.on_wait = keep
                    inst.sync_info = si
                    for k, d in enumerate(new_insts):
                        insts.insert(i + k, d)
                    i += len(new_insts)
                    n_fixed += 1
                i += 1
    return n_fixed



def build_program():
    nc = bass.Bass("TRN2", target_bir_lowering=False, debug=False)

    def din(name, shape, dt=F32):
        return nc.dram_tensor(name, shape, dt, kind="ExternalInput").ap()

    x_in = din("x_shard", [C, H, W])
    wk2 = din("wk2", [128, 9, 128], BF16)     # key conv per-tap block-diag lhsT
    w1px2 = din("w1px2", [128, 64], BF16)     # e1 x-part lhsT (both halves)
    w1pk2 = din("w1pk2", [128, 64], BF16)     # e1 k-part lhsT
    w2p = din("w2p", [64, 72], BF16)          # e2 lhsT (rows 32:64 duplicate)
    wc2 = din("wc2", [128, 128], BF16)        # c1 block-diag lhsT
    be2 = din("b_e2", [72, 1])
    gnw = din("gn_w", [72, 1])
    gnb = din("gn_b", [72, 1])
    g18 = din("g18", [72, 8], BF16)           # group-mean matrix (1/18)
    b72m = din("b72m", [8, 72], BF16)         # group->row broadcast matrix
    wdu12 = din("wdu12", [128, 4], BF16)      # attention fc1 (1/NPIX folded)
    wdu22 = din("wdu22", [4, 128], BF16)      # attention fc2
    i128 = din("i128", [128, 128], BF16)      # identity (PE tap-sum)
    b128 = din("b128", [8, 128], BF16)        # group->channel broadcast
    gnwi = din("gnwi", [128, 9])              # gn_w per (channel, tap)
    gnbi = din("gnbi", [128, 9])              # gn_b per (channel, tap)
    out_d = nc.dram_tensor("out", [C, H, W // 2], F32, kind="ExternalOutput").ap()

    with tile.TileContext(nc) as tc:
        _build(tc, x_in, wk2, w1px2, w1pk2, w2p, wc2, be2, gnw, gnb,
               g18, b72m, wdu12, wdu22, i128, b128, gnwi, gnbi, out_d)

    _split_big_waits(nc)
    return nc


def _build(tc, x_in, wk2, w1px2, w1pk2, w2p, wc2, be2, gnw, gnb,
           g18, b72m, wdu12, wdu22, i128, b128, gnwi, gnbi, out_d):
    nc = tc.nc
    from contextlib import ExitStack

    ctx = ExitStack()
    with ctx:
        big = ctx.enter_context(tc.tile_pool(name="big", bufs=1))
        weights = ctx.enter_context(tc.tile_pool(name="weights", bufs=1))
        small = ctx.enter_context(tc.tile_pool(name="small", bufs=1))
        wbp = ctx.enter_context(tc.tile_pool(name="wbp", bufs=20))
        prp = ctx.enter_context(tc.tile_pool(name="prp", bufs=3))
        w1p_pool = ctx.enter_context(tc.tile_pool(name="w1c", bufs=3))
        otp = ctx.enter_context(tc.tile_pool(name="otp", bufs=3))
        # PSUM: kp(2) + e1(2) + e2T(1) + e2B(1) + small(2) = 8 banks;
        # the stage A pools are released before stage C (identity-sum banks)
        psm = tc.alloc_tile_pool(name="psm", bufs=2, space="PSUM")
        pk = tc.alloc_tile_pool(name="pk", bufs=2, space="PSUM")
        pe1 = tc.alloc_tile_pool(name="pe1", bufs=2, space="PSUM")
        pe2t = tc.alloc_tile_pool(name="pe2t", bufs=1, space="PSUM")
        pe2b = tc.alloc_tile_pool(name="pe2b", bufs=1, space="PSUM")

        # ---- resident buffers ----
        X2 = big.tile([128, PH2, PW], BF16)    # x halves, padded
        XV2 = big.tile([128, PH2, PW], BF16)   # c1 out halves, padded
        W72T = big.tile([72, NPIXH], BF16)     # e2+bias top half
        W72B = big.tile([72, NPIXH], BF16)     # e2+bias bottom half
        OUT2 = big.tile([128, NPIXH], BF16)    # local conv accumulator

        # ---- weights ----
        WK2 = weights.tile([128, 9, 128], BF16)
        nc.sync.dma_start(WK2[:], wk2[:])
        W1PX = weights.tile([128, 64], BF16)
        nc.sync.dma_start(W1PX[:], w1px2[:])
        W1PK = weights.tile([128, 64], BF16)
        nc.sync.dma_start(W1PK[:], w1pk2[:])
        W2P = weights.tile([64, 72], BF16)
        nc.sync.dma_start(W2P[:], w2p[:])
        WC2 = weights.tile([128, 128], BF16)
        nc.sync.dma_start(WC2[:], wc2[:])
        BE2 = weights.tile([72, 1], F32)
        nc.sync.dma_start(BE2[:], be2[:])
        GNW = weights.tile([72, 1], F32)
        nc.sync.dma_start(GNW[:], gnw[:])
        GNB = weights.tile([72, 1], F32)
        nc.sync.dma_start(GNB[:], gnb[:])
        G18 = weights.tile([72, 8], BF16)
        nc.sync.dma_start(G18[:], g18[:])
        B72 = weights.tile([8, 72], BF16)
        nc.sync.dma_start(B72[:], b72m[:])
        WDU1 = weights.tile([128, 4], BF16)
        nc.sync.dma_start(WDU1[:], wdu12[:])
        WDU2 = weights.tile([4, 128], BF16)
        nc.sync.dma_start(WDU2[:], wdu22[:])
        I128 = weights.tile([128, 128], BF16)
        nc.sync.dma_start(I128[:], i128[:])
        B128 = weights.tile([8, 128], BF16)
        nc.sync.dma_start(B128[:], b128[:])
        GNWI = weights.tile([128, 9], F32)
        nc.sync.dma_start(GNWI[:], gnwi[:])
        GNBI = weights.tile([128, 9], F32)
        nc.sync.dma_start(GNBI[:], gnbi[:])

        # ---- zero pads (only the pad regions) ----
        for buf in (X2, XV2):
            nc.gpsimd.memset(buf[:, :, 0:2], 0.0)
            nc.gpsimd.memset(buf[:, :, 130:132], 0.0)
            nc.gpsimd.memset(buf[0:64, 0:1, :], 0.0)
            nc.gpsimd.memset(buf[64:128, 65:66, :], 0.0)

        # ---- load x via gpsimd casting DMA (f32 HBM -> bf16 padded SBUF);
        # halo rows first (needed by the first chunks) ----
        nc.gpsimd.dma_start(X2[0:64, 65:66, 2:130], x_in[:, 64:65, :])
        nc.gpsimd.dma_start(X2[64:128, 0:1, 2:130], x_in[:, 63:64, :])
        for lq in range(NBQ):
            r0 = lq * RBQ
            nc.gpsimd.dma_start(X2[0:64, 1 + r0:1 + r0 + RBQ, 2:130],
                                x_in[:, r0:r0 + RBQ, :])
            nc.gpsimd.dma_start(X2[64:128, 1 + r0:1 + r0 + RBQ, 2:130],
                                x_in[:, 64 + r0:64 + r0 + RBQ, :])

        taps = [(di, dj) for di in range(3) for dj in range(3)]
        wbs = {}
        statsT = small.tile([72, NCH, 6], F32)
        statsB = small.tile([72, NCH, 6], F32)

        def img(buf, h0, nrows=RPC):
            return buf[:, 1 + h0:1 + h0 + nrows, 2:2 + W]

        def tapv(buf, h0, di, dj, nrows=RPC):
            return buf[:, h0 + di:h0 + di + nrows, 1 + dj:1 + dj + W]

        # ======== stage A: per-chunk convs (pairs of chunks) ========
        for qp in range(NCH // 2):
            qa, qb = 2 * qp, 2 * qp + 1
            pks = {}
            # key conv: tap-outer over the chunk pair
            for t in range(9):
                di, dj = taps[t]
                for q in (qa, qb):
                    if t == 0:
                        pks[q] = pk.tile([128, RPC, W], F32, tag="kp", name="pkq")
                    nc.tensor.matmul(
                        pks[q][:], WK2[:, t, :], tapv(X2, q * RPC, di, dj),
                        start=(t == 0), stop=(t == 8),
                    )
            k2s = {}
            for q in (qa, qb):
                k2s[q] = w1p_pool.tile([128, CH], BF16, tag="k2", name="k2q")
                nc.scalar.activation(
                    k2s[q][:].rearrange("p (a b) -> p a b", a=RPC),
                    pks[q][:], AF.Relu)
            # e1: two accumulating 64-contraction matmuls
            p1s = {}
            for q in (qa, qb):
                p1s[q] = pe1.tile([64, CH], F32, tag="e1", name="p1q")
                nc.tensor.matmul(
                    p1s[q][:],
                    W1PX[:], img(X2, q * RPC),
                    start=True, stop=False)
            for q in (qa, qb):
                nc.tensor.matmul(
                    p1s[q][:], W1PK[:], k2s[q][:], start=False, stop=True)
            w1cs = {}
            for q in (qa, qb):
                w1cs[q] = w1p_pool.tile([64, CH], BF16, tag="w1c", name="w1cq")
                nc.scalar.activation(w1cs[q][:], p1s[q][:], AF.Relu)
            # e2 per half (+bias on the ACT copy out of psum)
            for q in (qa, qb):
                qs = slice(q * CH, (q + 1) * CH)
                pt = pe2t.tile([72, CH], F32, tag="e2t")
                nc.tensor.matmul(pt[:], W2P[0:32, :], w1cs[q][0:32, :],
                                 start=True, stop=True)
                nc.scalar.activation(W72T[:, qs], pt[:], AF.Identity,
                                     bias=BE2[:])
                pb = pe2b.tile([72, CH], F32, tag="e2b")
                nc.tensor.matmul(pb[:], W2P[32:64, :], w1cs[q][32:64, :],
                                 start=True, stop=True)
                nc.scalar.activation(W72B[:, qs], pb[:], AF.Identity,
                                     bias=BE2[:])
            # c1 (reuses the key psum ring)
            for q in (qa, qb):
                pc = pk.tile([128, RPC, W], F32, tag="kp")
                nc.tensor.matmul(pc[:], WC2[:], img(X2, q * RPC),
                                 start=True, stop=True)
                nc.vector.tensor_copy(img(XV2, q * RPC), pc[:])
            # GN statistics (bn_stats free dim is capped at 512)
            for q in (qa, qb):
                qs = slice(q * CH, (q + 1) * CH)
                nc.vector.bn_stats(out=statsT[:, q, :], in_=W72T[:, qs])
                nc.vector.bn_stats(out=statsB[:, q, :], in_=W72B[:, qs])
            if qp == 0:
                # bottom half's upper halo = top half's chunk-0 output... no:
                # XV2[64:128, 0] (bottom halo) = xv image row 63 -> top half
                # local row 63 = chunk 15 (NOT ready yet).
                # XV2[0:64, 65] (top halo) = xv image row 64 -> bottom half
                # local row 0 = chunk 0 (ready now).
                nc.sync.dma_start(XV2[0:64, 65:66, 2:130], XV2[64:128, 1:2, 2:130])
            if qp == NCH // 2 - 1:
                # on the scalar queue: the sync queue may be stalled on wb
                # ring slots whose release needs stage C (which reads this)
                nc.scalar.dma_start(XV2[64:128, 0:1, 2:130], XV2[0:64, 64:65, 2:130])
            if qp % 2 == 1 and qp // 2 < 3:
                # raw kernel broadcasts for the completed big chunk (sync
                # hwdge; GN scale/bias is applied post-broadcast).  Only the
                # first two big chunks are prefetched raw; the last two are
                # broadcast after GN is applied directly on W72.
                bqd = qp // 2
                bs = slice(bqd * BQ, (bqd + 1) * BQ)
                for t in range(9 if bqd < 2 else 2):
                    wb = wbp.tile([128, BQ], BF16, tag="wb", name="wbt")
                    sT = W72T[:][t:72:9, bs].unsqueeze(1).broadcast_to([8, 8, BQ])
                    sB = W72B[:][t:72:9, bs].unsqueeze(1).broadcast_to([8, 8, BQ])
                    nc.sync.dma_start(wb[0:64, :], sT)
                    nc.sync.dma_start(wb[64:128, :], sB)
                    wbs[(bqd, t)] = wb

        # ======== stage B: GroupNorm scale/bias ========
        mvT = small.tile([72, 2], F32)
        nc.vector.bn_aggr(out=mvT[:], in_=statsT[:])
        mvB = small.tile([72, 2], F32)
        nc.vector.bn_aggr(out=mvB[:], in_=statsB[:])
        packT = small.tile([72, 2], BF16)
        packB = small.tile([72, 2], BF16)
        for mv, pack in ((mvT, packT), (mvB, packB)):
            rowq = small.tile([72, 1], F32, tag="rowq")
            nc.vector.tensor_mul(rowq[:], mv[:, 0:1], mv[:, 0:1])
            nc.vector.tensor_add(rowq[:], rowq[:], mv[:, 1:2])
            nc.vector.tensor_copy(pack[:, 0:1], mv[:, 0:1])
            nc.vector.tensor_copy(pack[:, 1:2], rowq[:])
        pg = psm.tile([128, 2], F32, tag="sp")
        nc.tensor.matmul(pg[0:8, :], G18[:], packT[:], start=True, stop=False)
        nc.tensor.matmul(pg[0:8, :], G18[:], packB[:], start=False, stop=True)
        gm = small.tile([8, 2], F32)
        nc.vector.tensor_copy(gm[:], pg[0:8, :])
        msq = small.tile([8, 1], F32)
        nc.vector.tensor_mul(msq[:], gm[:, 0:1], gm[:, 0:1])
        v8 = small.tile([8, 1], F32)
        nc.vector.tensor_tensor(out=v8[:], in0=gm[:, 1:2], in1=msq[:],
                                op=ALU.subtract)
        eps8 = small.tile([8, 1], F32)
        nc.vector.memset(eps8[:], EPS)
        sd8 = small.tile([8, 1], F32)
        nc.scalar.activation(sd8[:], v8[:], AF.Sqrt, bias=eps8[:])
        rstd8 = small.tile([8, 2], F32)
        nc.vector.reciprocal(rstd8[:, 0:1], sd8[:])
        nc.vector.tensor_copy(rstd8[:, 1:2], gm[:, 0:1])
        rstd8b = small.tile([8, 2], BF16)
        nc.vector.tensor_copy(rstd8b[:], rstd8[:])
        # broadcast (rstd, m) to channels (tap-independent), then build the
        # per-(channel, tap) scale/bias tables with wide DVE ops
        p128 = psm.tile([128, 2], F32, tag="sp")
        nc.tensor.matmul(p128[:], B128[:], rstd8b[:], start=True, stop=True)
        r128 = small.tile([128, 2], F32)
        nc.vector.tensor_copy(r128[:], p128[:])
        ABTa = small.tile([128, 9], F32)
        nc.vector.tensor_scalar(out=ABTa[:], in0=GNWI[:], scalar1=r128[:, 0:1],
                                scalar2=None, op0=ALU.mult)
        ABTb = small.tile([128, 9], F32)
        nc.vector.tensor_scalar(out=ABTb[:], in0=ABTa[:], scalar1=r128[:, 1:2],
                                scalar2=None, op0=ALU.mult)
        nc.vector.tensor_tensor(out=ABTb[:], in0=GNBI[:], in1=ABTb[:],
                                op=ALU.subtract)
        # per-row scale/bias for the in-place W72 GN-apply (bq2/3 source)
        p72 = psm.tile([128, 2], F32, tag="sp")
        nc.tensor.matmul(p72[0:72, :], B72[:], rstd8b[:], start=True, stop=True)
        rs72 = small.tile([72, 2], F32)
        nc.vector.tensor_copy(rs72[:], p72[0:72, :])
        a72 = small.tile([72, 1], F32)
        nc.vector.tensor_mul(a72[:], rs72[:, 0:1], GNW[:])
        b72 = small.tile([72, 1], F32)
        nc.vector.tensor_mul(b72[:], rs72[:, 1:2], a72[:])
        nc.vector.tensor_tensor(out=b72[:], in0=GNB[:], in1=b72[:],
                                op=ALU.subtract)

        pe2b.release()
        pe2t.release()
        pe1.release()
        pk.release()
        psm.release()
        pcs = tc.alloc_tile_pool(name="pcs", bufs=8, space="PSUM")

        # ======== stage C: normalize prefetched kernels + local conv ========
        # products on DVE; the tap-sum runs on the (idle) PE as accumulating
        # identity matmuls into PSUM (f32), drained by DVE copies w/ row-sums
        ys4 = small.tile([128, NBQ], F32)
        for bq in range(NBQ):
            r0 = bq * RBQ
            banks = {}
            pa = None
            for t in range(9):
                di, dj = taps[t]
                wb = wbs.pop((bq, t))
                if bq >= 2 and not (bq == 2 and t < 2):
                    pass  # already normalized at the source
                elif bq == 1 or t in (0, 1, 2, 3, 7):
                    # bq1 stays off gpsimd: the gp queue carries the bq3
                    # broadcast issues which stall on wb ring slots that
                    # bq1's products release
                    nc.scalar.activation(wb[:], wb[:], AF.Identity,
                                         bias=ABTb[:, t:t + 1], scale=ABTa[:, t:t + 1])
                else:
                    nc.gpsimd.tensor_scalar(
                        out=wb[:], in0=wb[:], scalar1=ABTa[:, t:t + 1],
                        scalar2=ABTb[:, t:t + 1], op0=ALU.mult, op1=ALU.add)
                wbv = wb[:].rearrange("p (a b) -> p a b", a=RBQ)
                xs = tapv(XV2, r0, di, dj, nrows=RBQ)
                if t < 6:
                    # PE-summed taps
                    p = prp.tile([128, BQ], BF16, tag="p")
                    pv = p[:].rearrange("p (a b) -> p a b", a=RBQ)
                    nc.vector.tensor_mul(pv, xs, wbv)
                    for c in range(4):
                        if t == 0:
                            banks[c] = pcs.tile([128, CH], F32, tag="cb",
                                                name="cbank")
                        nc.tensor.matmul(
                            banks[c][:], I128[:], p[:, c * CH:(c + 1) * CH],
                            start=(t == 0), stop=(t == 5))
                elif t == 6:
                    pa = prp.tile([128, BQ], BF16, tag="pa", name="pacc", bufs=2)
                    nc.vector.tensor_mul(
                        pa[:].rearrange("p (a b) -> p a b", a=RBQ), xs, wbv)
                else:
                    p = prp.tile([128, BQ], BF16, tag="p")
                    pv = p[:].rearrange("p (a b) -> p a b", a=RBQ)
                    nc.vector.tensor_mul(pv, xs, wbv)
                    nc.vector.tensor_add(pa[:], pa[:], p[:])
            for c in range(4):
                q = bq * 4 + c
                nc.vector.tensor_tensor(
                    out=OUT2[:, q * CH:(q + 1) * CH], in0=banks[c][:],
                    in1=pa[:, c * CH:(c + 1) * CH], op=ALU.add)
            nc.vector.tensor_reduce(
                ys4[:, bq:bq + 1], OUT2[:, bq * BQ:(bq + 1) * BQ],
                axis=mybir.AxisListType.X, op=ALU.add)
            if bq == 0:
                # GN-apply in place on the second half of W72, then broadcast
                # the already-normalized kernels for big chunks 2-3 (overlaps
                # with big-chunk-1 compute)
                half = slice(2 * BQ, 4 * BQ)
                nc.scalar.activation(W72T[:, half], W72T[:, half], AF.Identity,
                                     bias=b72[:], scale=a72[:])
                nc.gpsimd.tensor_scalar(out=W72B[:, half], in0=W72B[:, half],
                                        scalar1=a72[:], scalar2=b72[:],
                                        op0=ALU.mult, op1=ALU.add)
                for bqd in (2, 3):
                    bs2 = slice(bqd * BQ, (bqd + 1) * BQ)
                    for t in range(2 if bqd == 2 else 0, 9):
                        wb = wbp.tile([128, BQ], BF16, tag="wb", name="wbt")
                        sT = W72T[:][t:72:9, bs2].unsqueeze(1).broadcast_to([8, 8, BQ])
                        sB = W72B[:][t:72:9, bs2].unsqueeze(1).broadcast_to([8, 8, BQ])
                        nc.sync.dma_start(wb[0:64, :], sT)
                        nc.sync.dma_start(wb[64:128, :], sB)
                        wbs[(bqd, t)] = wb

        pcs.release()
        psm2 = tc.alloc_tile_pool(name="psm2", bufs=2, space="PSUM")

        # ======== stage D: channel attention + store ========
        ysum = small.tile([128, 1], F32)
        nc.vector.tensor_reduce(ysum[:], ys4[:], axis=mybir.AxisListType.X,
                                op=ALU.add)
        ysb = small.tile([128, 1], BF16)
        nc.vector.tensor_copy(ysb[:], ysum[:])
        pa1 = psm2.tile([128, 2], F32, tag="sp")
        nc.tensor.matmul(pa1[0:4, 0:1], WDU1[:], ysb[:], start=True, stop=True)
        y1 = small.tile([4, 1], BF16)
        nc.scalar.activation(y1[:], pa1[0:4, 0:1], AF.Relu)
        pa2 = psm2.tile([128, 2], F32, tag="sp")
        nc.tensor.matmul(pa2[:, 0:1], WDU2[:], y1[:], start=True, stop=True)
        yatt = small.tile([128, 1], F32)
        nc.scalar.activation(yatt[:], pa2[:, 0:1], AF.Sigmoid)

        for hb in range(NBQ * 2):
            r0 = hb * (RBQ // 2)
            bs = slice(hb * (BQ // 2), (hb + 1) * (BQ // 2))
            ot = otp.tile([128, RBQ // 2, W], BF16, tag="ot")
            if hb % 2 == 0:
                nc.scalar.activation(
                    ot[:].rearrange("p a b -> p (a b)"), OUT2[:, bs],
                    AF.Identity, scale=yatt[:])
            else:
                nc.vector.tensor_scalar_mul(
                    ot[:].rearrange("p a b -> p (a b)"), OUT2[:, bs], yatt[:])
            otf = ot[:].bitcast(F32)
            nc.sync.dma_start(out_d[:, r0:r0 + RBQ // 2, :], otf[0:64])
            nc.scalar.dma_start(out_d[:, 64 + r0:64 + r0 + RBQ // 2, :], otf[64:128])
        psm2.release()


def _b128():
    m = np.zeros((8, 128), np.float32)
    for c in range(128):
        m[(c % 64) // 8, c] = 1.0
    return m


def _gn_per_tap(v):
    v = np.asarray(v, np.float32).reshape(72)
    out = np.zeros((128, 9), np.float32)
    for c in range(128):
        g = (c % 64) // 8
        for t in range(9):
            out[c, t] = v[g * 9 + t]
    return out


def prep_weights(w_key, w_e1, w_e2, b_e2, gn_w, gn_b, w_c1, w_du1, w_du2):
    import ml_dtypes

    bf = ml_dtypes.bfloat16
    # key conv: per tap block-diag [9, 128, 128]
    wk2 = np.zeros((128, 9, 128), np.float32)
    for t in range(9):
        di, dj = t // 3, t % 3
        base = np.zeros((64, 64), np.float32)
        for o in range(64):
            g = o // 8
            for j in range(8):
                base[g * 8 + j, o] = w_key[o, j, di, dj]
        wk2[0:64, t, 0:64] = base
        wk2[64:128, t, 64:128] = base

    # e1: split x/k parts with halves stacking
    w1px = np.zeros((64, 32), np.float32)
    w1pk = np.zeros((64, 32), np.float32)
    for r in range(64):
        qx = 2 * r       # x channel r in qk interleave
        qk_ = 2 * r + 1  # k channel r
        if qx < 64:
            w1px[r, 0:16] = w_e1[0:16, qx, 0, 0]
        else:
            w1px[r, 16:32] = w_e1[16:32, qx - 64, 0, 0]
        if qk_ < 64:
            w1pk[r, 0:16] = w_e1[0:16, qk_, 0, 0]
        else:
            w1pk[r, 16:32] = w_e1[16:32, qk_ - 64, 0, 0]
    w1px2 = np.zeros((128, 64), np.float32)
    w1px2[0:64, 0:32] = w1px
    w1px2[64:128, 32:64] = w1px
    w1pk2 = np.zeros((128, 64), np.float32)
    w1pk2[0:64, 0:32] = w1pk
    w1pk2[64:128, 32:64] = w1pk

    # e2 (per half): [32, 72], duplicated to rows 32:64 for the
    # bottom-half matmul (fmap and weights must share a base partition)
    w2p1 = np.zeros((32, 72), np.float32)
    for j in range(32):
        if j < 16:
            w2p1[j, 0:36] = w_e2[0:36, j, 0, 0]
        else:
            w2p1[j, 36:72] = w_e2[36:72, j - 16, 0, 0]
    w2p = np.vstack([w2p1, w2p1])

    # c1 block-diag
    wc1 = np.zeros((64, 64), np.float32)
    for o in range(64):
        if o < 32:
            wc1[0:32, o] = w_c1[o, :, 0, 0]
        else:
            wc1[32:64, o] = w_c1[o, :, 0, 0]
    wc2 = np.zeros((128, 128), np.float32)
    wc2[0:64, 0:64] = wc1
    wc2[64:128, 64:128] = wc1

    g18 = np.zeros((72, 8), np.float32)
    for r in range(72):
        g18[r, r // 9] = 1.0 / 18.0
    b72m = np.zeros((8, 72), np.float32)
    for r in range(72):
        b72m[r // 9, r] = 1.0

    wdu1 = (w_du1[:, :, 0, 0].T / float(NPIX)).astype(np.float32)  # [64, 4]
    wdu12 = np.vstack([wdu1, wdu1])                                # [128, 4]
    wdu2 = w_du2[:, :, 0, 0].T.astype(np.float32)                  # [4, 64]
    wdu22 = np.hstack([wdu2, wdu2])                                # [4, 128]

    return {
        "wk2": wk2.astype(bf),
        "w1px2": w1px2.astype(bf),
        "w1pk2": w1pk2.astype(bf),
        "w2p": w2p.astype(bf),
        "wc2": wc2.astype(bf),
        "b_e2": b_e2.reshape(72, 1).astype(np.float32),
        "gn_w": gn_w.reshape(72, 1).astype(np.float32),
        "gn_b": gn_b.reshape(72, 1).astype(np.float32),
        "g18": g18.astype(bf),
        "b72m": b72m.astype(bf),
        "wdu12": wdu12.astype(bf),
        "wdu22": wdu22.astype(bf),
        "i128": np.eye(128, dtype=np.float32).astype(bf),
        "b128": _b128().astype(bf),
        "gnwi": _gn_per_tap(gn_w),
        "gnbi": _gn_per_tap(gn_b),
    }


def emulate(x, wm):
    """Numpy emulation of the on-core dataflow (f32; validates index maps)."""
    def half_stack(a):  # [64, 128, 128] -> [128, 64, 128]
        return np.concatenate([a[:, 0:64], a[:, 64:128]], axis=0)

    xs = half_stack(x)  # [128, 64, 128]
    # padded X2
    X2 = np.zeros((128, PH2, PW), np.float32)
    X2[:, 1:65, 2:130] = xs
    X2[0:64, 65, 2:130] = x[:, 64]
    X2[64:128, 0, 2:130] = x[:, 63]

    def tapv(buf, di, dj):  # full-image tap view [128, 64, 128]
        return buf[:, di:di + 64, 1 + dj:1 + dj + 128]

    # key conv
    pk = np.zeros((128, 64, 128), np.float32)
    for t in range(9):
        di, dj = t // 3, t % 3
        pk += np.einsum('io,ihw->ohw', wm["wk2"][:, t, :].astype(np.float32),
                        tapv(X2, di, dj))
    K2 = np.maximum(pk, 0)
    # e1
    p1 = (np.einsum('io,ihw->ohw', wm["w1px2"].astype(np.float32), X2[:, 1:65, 2:130])
          + np.einsum('io,ihw->ohw', wm["w1pk2"].astype(np.float32), K2))
    W1c = np.maximum(p1, 0)  # [64, 64, 128]
    # e2 per half + bias
    w2p = wm["w2p"][0:32].astype(np.float32)
    be2 = wm["b_e2"].astype(np.float32)
    W72T = np.einsum('io,ihw->ohw', w2p, W1c[0:32]) + be2[:, None]
    W72B = np.einsum('io,ihw->ohw', w2p, W1c[32:64]) + be2[:, None]
    # GN over both halves
    cat = np.stack([W72T, W72B], axis=1).reshape(8, 18, 64, 128)
    m = cat.mean(axis=(1, 2, 3), keepdims=True)
    v = cat.var(axis=(1, 2, 3), keepdims=True)
    a = (wm["gn_w"].astype(np.float32).reshape(8, 9, 1, 1, 1)
         / np.sqrt(v[:, None, 0] + EPS)[..., None].transpose(0, 1, 2, 3, 4)[:, :, 0:1])
    # simpler: compute rstd per group then per row
    rstd = 1.0 / np.sqrt(v.reshape(8) + EPS)
    mg = m.reshape(8)
    gw = wm["gn_w"].astype(np.float32).reshape(72)
    gb = wm["gn_b"].astype(np.float32).reshape(72)
    alpha = gw * rstd[np.arange(72) // 9]
    beta = gb - mg[np.arange(72) // 9] * alpha
    W72T = W72T * alpha[:, None, None] + beta[:, None, None]
    W72B = W72B * alpha[:, None, None] + beta[:, None, None]
    # c1
    pc = np.einsum('io,ihw->ohw', wm["wc2"].astype(np.float32), X2[:, 1:65, 2:130])
    XV2 = np.zeros((128, PH2, PW), np.float32)
    XV2[:, 1:65, 2:130] = pc
    XV2[0:64, 65, 2:130] = pc[64:128, 0]
    XV2[64:128, 0, 2:130] = pc[0:64, 63]
    # local conv
    OUT2 = np.zeros((128, 64, 128), np.float32)
    gidx = np.arange(128) // 8 * 9  # base row per channel (mod 72 within half)
    for t in range(9):
        di, dj = t // 3, t % 3
        xsv = tapv(XV2, di, dj)
        wbT = W72T[(np.arange(64) // 8) * 9 + t]
        wbB = W72B[(np.arange(64) // 8) * 9 + t]
        wb = np.concatenate([wbT, wbB], axis=0)
        OUT2 += xsv * wb
    # attention
    ysum = OUT2.sum(axis=(1, 2))
    y = ysum @ (wm["wdu12"].astype(np.float32))  # includes both halves + 1/NPIX
    y = np.maximum(y, 0)
    y = y @ wm["wdu22"].astype(np.float32)
    y = 1.0 / (1.0 + np.exp(-y))
    OUT2 = OUT2 * y[:, None, None]
    out = np.concatenate([OUT2[0:64], OUT2[64:128]], axis=1)
    return out


_PROGRAM_CACHE = {}


def _get_program():
    if "nc" not in _PROGRAM_CACHE:
        _PROGRAM_CACHE["nc"] = build_program()
    return _PROGRAM_CACHE["nc"]


def run_on_cores(inputs, trace=False):
    nc = _get_program()
    x = np.asarray(inputs["x"], np.float32)
    wmaps = prep_weights(
        np.asarray(inputs["w_key"], np.float32),
        np.asarray(inputs["w_e1"], np.float32),
        np.asarray(inputs["w_e2"], np.float32),
        np.asarray(inputs["b_e2"], np.float32),
        np.asarray(inputs["gn_w"], np.float32),
        np.asarray(inputs["gn_b"], np.float32),
        np.asarray(inputs["w_c1"], np.float32),
        np.asarray(inputs["w_du1"], np.float32),
        np.asarray(inputs["w_du2"], np.float32),
    )
    in_maps = []
    for b in range(8):
        m = {"x_shard": np.ascontiguousarray(x[b])}
        m.update(wmaps)
        in_maps.append(m)
    res = run_bass_kernel_spmd(nc, in_maps, core_ids=list(range(8)), trace=trace)
    import ml_dtypes
    outs = []
    for b in range(8):
        raw = np.ascontiguousarray(np.asarray(res.results[b]["out"], np.float32))
        bf = raw.view(ml_dtypes.bfloat16).reshape(C, H, W)
        outs.append(bf.astype(np.float32))
    out = np.stack(outs, axis=0)
    return out, res


def kernel(**inputs) -> np.ndarray:
    out, _ = run_on_cores(inputs, trace=False)
    return out.astype(np.float32)


# revision 3
# speedup vs baseline: 1.0844x; 1.0844x over previous
"""Trainium2 Bass kernel v2 for nn_CALayer — halves-stacked layout.

Data parallel over batch B=8 across 8 cores; within a core the image's
two row-halves (rows 0-63 / 64-127) are stacked on SBUF partitions
0-63 / 64-127.  All 64-channel convs become 128-wide block-diagonal
matmuls (half the PE passes of v1), and all element-wise work runs at
full 128-lane DVE width.  The per-pixel kernel broadcast (group row ->
8 channels) runs on the DMA engines via stride-0 access patterns
instead of PE matmuls.
"""

import numpy as np

import concourse.bass as bass
import concourse.tile as tile
from concourse import mybir
from concourse.bass_utils import run_bass_kernel_spmd

F32 = mybir.dt.float32
BF16 = mybir.dt.bfloat16

H = 128
W = 128
C = 64
HH = 64          # rows per half
NPIX = H * W
NPIXH = HH * W   # 8192 pixels per half
PH2 = 66         # padded rows per half (1 + 64 + 1)
PW = 132         # padded width
CH = 512         # psum chunk columns (4 rows per half)
NCH = NPIXH // CH    # 16 chunks
RPC = CH // W        # 4 rows per chunk
BQ = 2048        # big chunk for stage C / loads (16 rows per half)
NBQ = NPIXH // BQ    # 4
RBQ = BQ // W        # 16
EPS = 1e-5

AF = mybir.ActivationFunctionType
ALU = mybir.AluOpType


def _split_big_waits(nc, max_waits=1):
    """walrus CTRL codegen accepts only one sem wait per instruction; move
    extra waits onto Drain instructions inserted just before."""
    from concourse import mybir as _mybir
    n_fixed = 0
    for fn in nc.m.functions:
        for bb in fn.blocks:
            insts = bb.instructions
            i = 0
            while i < len(insts):
                inst = insts[i]
                si = inst.sync_info
                if si is not None and si.on_wait and len(si.on_wait) > max_waits:
                    waits = list(si.on_wait)
                    keep = waits[-max_waits:]
                    extra = waits[:-max_waits]
                    new_insts = []
                    for j in range(0, len(extra), max_waits):
                        chunk = extra[j : j + max_waits]
                        d = _mybir.InstDrain(
                            name=f"{inst.name}-waitsplit{j}", ins=[], outs=[]
                        )
                        d.engine = inst.engine
                        d.sync_info = _mybir.SyncInfo(on_wait=chunk, on_update=[])
                        new_insts.append(d)
                    si.on_wait = keep
                    inst.sync_info = si
                    for k, d in enumerate(new_insts):
                        insts.insert(i + k, d)
                    i += len(new_insts)
                    n_fixed += 1
                i += 1
    return n_fixed



def build_program():
    nc = bass.Bass("TRN2", target_bir_lowering=False, debug=False)

    def din(name, shape, dt=F32):
        return nc.dram_tensor(name, shape, dt, kind="ExternalInput").ap()

    x_in = din("x_shard", [C, H, W])
    wk2 = din("wk2", [128, 9, 128], BF16)     # key conv per-tap block-diag lhsT
    w1px2 = din("w1px2", [128, 64], BF16)     # e1 x-part lhsT (both halves)
    w1pk2 = din("w1pk2", [128, 64], BF16)     # e1 k-part lhsT
    w2p = din("w2p", [64, 72], BF16)          # e2 lhsT (rows 32:64 duplicate)
    wc2 = din("wc2", [128, 128], BF16)        # c1 block-diag lhsT
    be2 = din("b_e2", [72, 1])
    gnw = din("gn_w", [72, 1])
    gnb = din("gn_b", [72, 1])
    g18 = din("g18", [72, 8], BF16)           # group-mean matrix (1/18)
    b72m = din("b72m", [8, 72], BF16)         # group->row broadcast matrix
    wdu12 = din("wdu12", [128, 4], BF16)      # attention fc1 (1/NPIX folded)
    wdu22 = din("wdu22", [4, 128], BF16)      # attention fc2
    i128 = din("i128", [128, 128], BF16)      # identity (PE tap-sum)
    b128 = din("b128", [8, 128], BF16)        # group->channel broadcast
    gnwi = din("gnwi", [128, 9])              # gn_w per (channel, tap)
    gnbi = din("gnbi", [128, 9])              # gn_b per (channel, tap)
    out_d = nc.dram_tensor("out", [C, H, W // 2], F32, kind="ExternalOutput").ap()

    with tile.TileContext(nc) as tc:
        _build(tc, x_in, wk2, w1px2, w1pk2, w2p, wc2, be2, gnw, gnb,
               g18, b72m, wdu12, wdu22, i128, b128, gnwi, gnbi, out_d)

    _split_big_waits(nc)
    return nc


def _build(tc, x_in, wk2, w1px2, w1pk2, w2p, wc2, be2, gnw, gnb,
           g18, b72m, wdu12, wdu22, i128, b128, gnwi, gnbi, out_d):
    nc = tc.nc
    from contextlib import ExitStack

    ctx = ExitStack()
    with ctx:
        big = ctx.enter_context(tc.tile_pool(name="big", bufs=1))
        weights = ctx.enter_context(tc.tile_pool(name="weights", bufs=1))
        small = ctx.enter_context(tc.tile_pool(name="small", bufs=1))
        wbp = ctx.enter_context(tc.tile_pool(name="wbp", bufs=20))
        prp = ctx.enter_context(tc.tile_pool(name="prp", bufs=3))
        w1p_pool = ctx.enter_context(tc.tile_pool(name="w1c", bufs=3))
        otp = ctx.enter_context(tc.tile_pool(name="otp", bufs=3))
        # PSUM: kp(2) + e1(2) + e2T(1) + e2B(1) + small(2) = 8 banks;
        # the stage A pools are released before stage C (identity-sum banks)
        psm = tc.alloc_tile_pool(name="psm", bufs=2, space="PSUM")
        pk = tc.alloc_tile_pool(name="pk", bufs=2, space="PSUM")
        pe1 = tc.alloc_tile_pool(name="pe1", bufs=2, space="PSUM")
        pe2t = tc.alloc_tile_pool(name="pe2t", bufs=1, space="PSUM")
        pe2b = tc.alloc_tile_pool(name="pe2b", bufs=1, space="PSUM")

        # ---- resident buffers ----
        X2 = big.tile([128, PH2, PW], BF16)    # x halves, padded
        XV2 = big.tile([128, PH2, PW], BF16)   # c1 out halves, padded
        W72T = big.tile([72, NPIXH], BF16)     # e2+bias top half
        W72B = big.tile([72, NPIXH], BF16)     # e2+bias bottom half
        OUT2 = big.tile([128, NPIXH], BF16)    # local conv accumulator

        # ---- weights ----
        WK2 = weights.tile([128, 9, 128], BF16)
        nc.sync.dma_start(WK2[:], wk2[:])
        W1PX = weights.tile([128, 64], BF16)
        nc.sync.dma_start(W1PX[:], w1px2[:])
        W1PK = weights.tile([128, 64], BF16)
        nc.sync.dma_start(W1PK[:], w1pk2[:])
        W2P = weights.tile([64, 72], BF16)
        nc.sync.dma_start(W2P[:], w2p[:])
        WC2 = weights.tile([128, 128], BF16)
        nc.sync.dma_start(WC2[:], wc2[:])
        BE2 = weights.tile([72, 1], F32)
        nc.sync.dma_start(BE2[:], be2[:])
        GNW = weights.tile([72, 1], F32)
        nc.sync.dma_start(GNW[:], gnw[:])
        GNB = weights.tile([72, 1], F32)
        nc.sync.dma_start(GNB[:], gnb[:])
        G18 = weights.tile([72, 8], BF16)
        nc.sync.dma_start(G18[:], g18[:])
        B72 = weights.tile([8, 72], BF16)
        nc.sync.dma_start(B72[:], b72m[:])
        WDU1 = weights.tile([128, 4], BF16)
        nc.sync.dma_start(WDU1[:], wdu12[:])
        WDU2 = weights.tile([4, 128], BF16)
        nc.sync.dma_start(WDU2[:], wdu22[:])
        I128 = weights.tile([128, 128], BF16)
        nc.sync.dma_start(I128[:], i128[:])
        B128 = weights.tile([8, 128], BF16)
        nc.sync.dma_start(B128[:], b128[:])
        GNWI = weights.tile([128, 9], F32)
        nc.sync.dma_start(GNWI[:], gnwi[:])
        GNBI = weights.tile([128, 9], F32)
        nc.sync.dma_start(GNBI[:], gnbi[:])

        # ---- zero pads (only the pad regions) ----
        for buf in (X2, XV2):
            nc.gpsimd.memset(buf[:, :, 0:2], 0.0)
            nc.gpsimd.memset(buf[:, :, 130:132], 0.0)
            nc.gpsimd.memset(buf[0:64, 0:1, :], 0.0)
            nc.gpsimd.memset(buf[64:128, 65:66, :], 0.0)

        # ---- load x via gpsimd casting DMA (f32 HBM -> bf16 padded SBUF);
        # halo rows first (needed by the first chunks) ----
        nc.gpsimd.dma_start(X2[0:64, 65:66, 2:130], x_in[:, 64:65, :])
        nc.gpsimd.dma_start(X2[64:128, 0:1, 2:130], x_in[:, 63:64, :])
        for lq in range(NBQ):
            r0 = lq * RBQ
            nc.gpsimd.dma_start(X2[0:64, 1 + r0:1 + r0 + RBQ, 2:130],
                                x_in[:, r0:r0 + RBQ, :])
            nc.gpsimd.dma_start(X2[64:128, 1 + r0:1 + r0 + RBQ, 2:130],
                                x_in[:, 64 + r0:64 + r0 + RBQ, :])

        taps = [(di, dj) for di in range(3) for dj in range(3)]
        wbs = {}
        statsT = small.tile([72, NCH, 6], F32)
        statsB = small.tile([72, NCH, 6], F32)

        def img(buf, h0, nrows=RPC):
            return buf[:, 1 + h0:1 + h0 + nrows, 2:2 + W]

        def tapv(buf, h0, di, dj, nrows=RPC):
            return buf[:, h0 + di:h0 + di + nrows, 1 + dj:1 + dj + W]

        # ======== stage A: per-chunk convs (pairs of chunks) ========
        for qp in range(NCH // 2):
            qa, qb = 2 * qp, 2 * qp + 1
            pks = {}
            # key conv: tap-outer over the chunk pair
            for t in range(9):
                di, dj = taps[t]
                for q in (qa, qb):
                    if t == 0:
                        pks[q] = pk.tile([128, RPC, W], F32, tag="kp", name="pkq")
                    nc.tensor.matmul(
                        pks[q][:], WK2[:, t, :], tapv(X2, q * RPC, di, dj),
                        start=(t == 0), stop=(t == 8),
                    )
            k2s = {}
            for q in (qa, qb):
                k2s[q] = w1p_pool.tile([128, CH], BF16, tag="k2", name="k2q")
                nc.scalar.activation(
                    k2s[q][:].rearrange("p (a b) -> p a b", a=RPC),
                    pks[q][:], AF.Relu)
            # e1: two accumulating 64-contraction matmuls
            p1s = {}
            for q in (qa, qb):
                p1s[q] = pe1.tile([64, CH], F32, tag="e1", name="p1q")
                nc.tensor.matmul(
                    p1s[q][:],
                    W1PX[:], img(X2, q * RPC),
                    start=True, stop=False)
            for q in (qa, qb):
                nc.tensor.matmul(
                    p1s[q][:], W1PK[:], k2s[q][:], start=False, stop=True)
            w1cs = {}
            for q in (qa, qb):
                w1cs[q] = w1p_pool.tile([64, CH], BF16, tag="w1c", name="w1cq")
                nc.scalar.activation(w1cs[q][:], p1s[q][:], AF.Relu)
            # e2 per half (+bias on the ACT copy out of psum)
            for q in (qa, qb):
                qs = slice(q * CH, (q + 1) * CH)
                pt = pe2t.tile([72, CH], F32, tag="e2t")
                nc.tensor.matmul(pt[:], W2P[0:32, :], w1cs[q][0:32, :],
                                 start=True, stop=True)
                nc.scalar.activation(W72T[:, qs], pt[:], AF.Identity,
                                     bias=BE2[:])
                pb = pe2b.tile([72, CH], F32, tag="e2b")
                nc.tensor.matmul(pb[:], W2P[32:64, :], w1cs[q][32:64, :],
                                 start=True, stop=True)
                nc.scalar.activation(W72B[:, qs], pb[:], AF.Identity,
                                     bias=BE2[:])
            # c1 (reuses the key psum ring)
            for q in (qa, qb):
                pc = pk.tile([128, RPC, W], F32, tag="kp")
                nc.tensor.matmul(pc[:], WC2[:], img(X2, q * RPC),
                                 start=True, stop=True)
                nc.vector.tensor_copy(img(XV2, q * RPC), pc[:])
            # GN statistics (bn_stats free dim is capped at 512)
            for q in (qa, qb):
                qs = slice(q * CH, (q + 1) * CH)
                nc.vector.bn_stats(out=statsT[:, q, :], in_=W72T[:, qs])
                nc.vector.bn_stats(out=statsB[:, q, :], in_=W72B[:, qs])
            if qp == 0:
                # bottom half's upper halo = top half's chunk-0 output... no:
                # XV2[64:128, 0] (bottom halo) = xv image row 63 -> top half
                # local row 63 = chunk 15 (NOT ready yet).
                # XV2[0:64, 65] (top halo) = xv image row 64 -> bottom half
                # local row 0 = chunk 0 (ready now).
                nc.sync.dma_start(XV2[0:64, 65:66, 2:130], XV2[64:128, 1:2, 2:130])
            if qp == NCH // 2 - 1:
                # on the scalar queue: the sync queue may be stalled on wb
                # ring slots whose release needs stage C (which reads this)
                nc.scalar.dma_start(XV2[64:128, 0:1, 2:130], XV2[0:64, 64:65, 2:130])
            if qp % 2 == 1 and qp // 2 < 3:
                # raw kernel broadcasts for the completed big chunk (sync
                # hwdge; GN scale/bias is applied post-broadcast).  Only the
                # first two big chunks are prefetched raw; the last two are
                # broadcast after GN is applied directly on W72.
                bqd = qp // 2
                bs = slice(bqd * BQ, (bqd + 1) * BQ)
                for t in range(9):
                    wb = wbp.tile([128, BQ], BF16, tag="wb", name="wbt")
                    sT = W72T[:][t:72:9, bs].unsqueeze(1).broadcast_to([8, 8, BQ])
                    sB = W72B[:][t:72:9, bs].unsqueeze(1).broadcast_to([8, 8, BQ])
                    nc.sync.dma_start(wb[0:64, :], sT)
                    nc.sync.dma_start(wb[64:128, :], sB)
                    wbs[(bqd, t)] = wb

        # ======== stage B: GroupNorm scale/bias ========
        mvT = small.tile([72, 2], F32)
        nc.vector.bn_aggr(out=mvT[:], in_=statsT[:])
        mvB = small.tile([72, 2], F32)
        nc.vector.bn_aggr(out=mvB[:], in_=statsB[:])
        packT = small.tile([72, 2], BF16)
        packB = small.tile([72, 2], BF16)
        for mv, pack in ((mvT, packT), (mvB, packB)):
            rowq = small.tile([72, 1], F32, tag="rowq")
            nc.vector.tensor_mul(rowq[:], mv[:, 0:1], mv[:, 0:1])
            nc.vector.tensor_add(rowq[:], rowq[:], mv[:, 1:2])
            nc.vector.tensor_copy(pack[:, 0:1], mv[:, 0:1])
            nc.vector.tensor_copy(pack[:, 1:2], rowq[:])
        pg = psm.tile([128, 2], F32, tag="sp")
        nc.tensor.matmul(pg[0:8, :], G18[:], packT[:], start=True, stop=False)
        nc.tensor.matmul(pg[0:8, :], G18[:], packB[:], start=False, stop=True)
        gm = small.tile([8, 2], F32)
        nc.vector.tensor_copy(gm[:], pg[0:8, :])
        msq = small.tile([8, 1], F32)
        nc.vector.tensor_mul(msq[:], gm[:, 0:1], gm[:, 0:1])
        v8 = small.tile([8, 1], F32)
        nc.vector.tensor_tensor(out=v8[:], in0=gm[:, 1:2], in1=msq[:],
                                op=ALU.subtract)
        eps8 = small.tile([8, 1], F32)
        nc.vector.memset(eps8[:], EPS)
        sd8 = small.tile([8, 1], F32)
        nc.scalar.activation(sd8[:], v8[:], AF.Sqrt, bias=eps8[:])
        rstd8 = small.tile([8, 2], F32)
        nc.vector.reciprocal(rstd8[:, 0:1], sd8[:])
        nc.vector.tensor_copy(rstd8[:, 1:2], gm[:, 0:1])
        rstd8b = small.tile([8, 2], BF16)
        nc.vector.tensor_copy(rstd8b[:], rstd8[:])
        # broadcast (rstd, m) to channels (tap-independent), then build the
        # per-(channel, tap) scale/bias tables with wide DVE ops
        p128 = psm.tile([128, 2], F32, tag="sp")
        nc.tensor.matmul(p128[:], B128[:], rstd8b[:], start=True, stop=True)
        r128 = small.tile([128, 2], F32)
        nc.vector.tensor_copy(r128[:], p128[:])
        ABTa = small.tile([128, 9], F32)
        nc.vector.tensor_scalar(out=ABTa[:], in0=GNWI[:], scalar1=r128[:, 0:1],
                                scalar2=None, op0=ALU.mult)
        ABTb = small.tile([128, 9], F32)
        nc.vector.tensor_scalar(out=ABTb[:], in0=ABTa[:], scalar1=r128[:, 1:2],
                                scalar2=None, op0=ALU.mult)
        nc.vector.tensor_tensor(out=ABTb[:], in0=GNBI[:], in1=ABTb[:],
                                op=ALU.subtract)
        # per-row scale/bias for the in-place W72 GN-apply (bq2/3 source)
        p72 = psm.tile([128, 2], F32, tag="sp")
        nc.tensor.matmul(p72[0:72, :], B72[:], rstd8b[:], start=True, stop=True)
        rs72 = small.tile([72, 2], F32)
        nc.vector.tensor_copy(rs72[:], p72[0:72, :])
        a72 = small.tile([72, 1], F32)
        nc.vector.tensor_mul(a72[:], rs72[:, 0:1], GNW[:])
        b72 = small.tile([72, 1], F32)
        nc.vector.tensor_mul(b72[:], rs72[:, 1:2], a72[:])
        nc.vector.tensor_tensor(out=b72[:], in0=GNB[:], in1=b72[:],
                                op=ALU.subtract)

        pe2b.release()
        pe2t.release()
        pe1.release()
        pk.release()
        psm.release()
        pcs = tc.alloc_tile_pool(name="pcs", bufs=8, space="PSUM")

        # ======== stage C: normalize prefetched kernels + local conv ========
        # products on DVE; the tap-sum runs on the (idle) PE as accumulating
        # identity matmuls into PSUM (f32), drained by DVE copies w/ row-sums
        ys4 = small.tile([128, NBQ], F32)
        for bq in range(NBQ):
            r0 = bq * RBQ
            banks = {}
            pa = None
            for t in range(9):
                di, dj = taps[t]
                wb = wbs.pop((bq, t))
                if bq == 3:
                    pass  # already normalized at the source
                elif bq == 1 or t in (0, 1, 2, 3, 7):
                    # bq1 stays off gpsimd: the gp queue carries the bq3
                    # broadcast issues which stall on wb ring slots that
                    # bq1's products release
                    nc.scalar.activation(wb[:], wb[:], AF.Identity,
                                         bias=ABTb[:, t:t + 1], scale=ABTa[:, t:t + 1])
                else:
                    nc.gpsimd.tensor_scalar(
                        out=wb[:], in0=wb[:], scalar1=ABTa[:, t:t + 1],
                        scalar2=ABTb[:, t:t + 1], op0=ALU.mult, op1=ALU.add)
                wbv = wb[:].rearrange("p (a b) -> p a b", a=RBQ)
                xs = tapv(XV2, r0, di, dj, nrows=RBQ)
                if t < 6:
                    # PE-summed taps
                    p = prp.tile([128, BQ], BF16, tag="p")
                    pv = p[:].rearrange("p (a b) -> p a b", a=RBQ)
                    nc.vector.tensor_mul(pv, xs, wbv)
                    for c in range(4):
                        if t == 0:
                            banks[c] = pcs.tile([128, CH], F32, tag="cb",
                                                name="cbank")
                        nc.tensor.matmul(
                            banks[c][:], I128[:], p[:, c * CH:(c + 1) * CH],
                            start=(t == 0), stop=(t == 5))
                elif t == 6:
                    pa = prp.tile([128, BQ], BF16, tag="pa", name="pacc", bufs=2)
                    nc.vector.tensor_mul(
                        pa[:].rearrange("p (a b) -> p a b", a=RBQ), xs, wbv)
                else:
                    p = prp.tile([128, BQ], BF16, tag="p")
                    pv = p[:].rearrange("p (a b) -> p a b", a=RBQ)
                    nc.vector.tensor_mul(pv, xs, wbv)
                    nc.vector.tensor_add(pa[:], pa[:], p[:])
            for c in range(4):
                q = bq * 4 + c
                nc.vector.tensor_tensor(
                    out=OUT2[:, q * CH:(q + 1) * CH], in0=banks[c][:],
                    in1=pa[:, c * CH:(c + 1) * CH], op=ALU.add)
            nc.vector.tensor_reduce(
                ys4[:, bq:bq + 1], OUT2[:, bq * BQ:(bq + 1) * BQ],
                axis=mybir.AxisListType.X, op=ALU.add)
            if bq == 0:
                # GN-apply in place on the second half of W72, then broadcast
                # the already-normalized kernels for big chunks 2-3 (overlaps
                # with big-chunk-1 compute)
                half = slice(3 * BQ, 4 * BQ)
                nc.scalar.activation(W72T[:, half], W72T[:, half], AF.Identity,
                                     bias=b72[:], scale=a72[:])
                nc.gpsimd.tensor_scalar(out=W72B[:, half], in0=W72B[:, half],
                                        scalar1=a72[:], scalar2=b72[:],
                                        op0=ALU.mult, op1=ALU.add)
                for bqd in (3,):
                    bs2 = slice(bqd * BQ, (bqd + 1) * BQ)
                    for t in range(9):
                        wb = wbp.tile([128, BQ], BF16, tag="wb", name="wbt")
                        sT = W72T[:][t:72:9, bs2].unsqueeze(1).broadcast_to([8, 8, BQ])
                        sB = W72B[:][t:72:9, bs2].unsqueeze(1).broadcast_to([8, 8, BQ])
                        nc.sync.dma_start(wb[0:64, :], sT)
                        nc.sync.dma_start(wb[64:128, :], sB)
                        wbs[(bqd, t)] = wb

        pcs.release()
        psm2 = tc.alloc_tile_pool(name="psm2", bufs=2, space="PSUM")

        # ======== stage D: channel attention + store ========
        ysum = small.tile([128, 1], F32)
        nc.vector.tensor_reduce(ysum[:], ys4[:], axis=mybir.AxisListType.X,
                                op=ALU.add)
        ysb = small.tile([128, 1], BF16)
        nc.vector.tensor_copy(ysb[:], ysum[:])
        pa1 = psm2.tile([128, 2], F32, tag="sp")
        nc.tensor.matmul(pa1[0:4, 0:1], WDU1[:], ysb[:], start=True, stop=True)
        y1 = small.tile([4, 1], BF16)
        nc.scalar.activation(y1[:], pa1[0:4, 0:1], AF.Relu)
        pa2 = psm2.tile([128, 2], F32, tag="sp")
        nc.tensor.matmul(pa2[:, 0:1], WDU2[:], y1[:], start=True, stop=True)
        yatt = small.tile([128, 1], F32)
        nc.scalar.activation(yatt[:], pa2[:, 0:1], AF.Sigmoid)

        for hb in range(NBQ * 2):
            r0 = hb * (RBQ // 2)
            bs = slice(hb * (BQ // 2), (hb + 1) * (BQ // 2))
            ot = otp.tile([128, RBQ // 2, W], BF16, tag="ot")
            if hb % 2 == 0:
                nc.scalar.activation(
                    ot[:].rearrange("p a b -> p (a b)"), OUT2[:, bs],
                    AF.Identity, scale=yatt[:])
            else:
                nc.vector.tensor_scalar_mul(
                    ot[:].rearrange("p a b -> p (a b)"), OUT2[:, bs], yatt[:])
            otf = ot[:].bitcast(F32)
            nc.sync.dma_start(out_d[:, r0:r0 + RBQ // 2, :], otf[0:64])
            nc.scalar.dma_start(out_d[:, 64 + r0:64 + r0 + RBQ // 2, :], otf[64:128])
        psm2.release()


def _b128():
    m = np.zeros((8, 128), np.float32)
    for c in range(128):
        m[(c % 64) // 8, c] = 1.0
    return m


def _gn_per_tap(v):
    v = np.asarray(v, np.float32).reshape(72)
    out = np.zeros((128, 9), np.float32)
    for c in range(128):
        g = (c % 64) // 8
        for t in range(9):
            out[c, t] = v[g * 9 + t]
    return out


def prep_weights(w_key, w_e1, w_e2, b_e2, gn_w, gn_b, w_c1, w_du1, w_du2):
    import ml_dtypes

    bf = ml_dtypes.bfloat16
    # key conv: per tap block-diag [9, 128, 128]
    wk2 = np.zeros((128, 9, 128), np.float32)
    for t in range(9):
        di, dj = t // 3, t % 3
        base = np.zeros((64, 64), np.float32)
        for o in range(64):
            g = o // 8
            for j in range(8):
                base[g * 8 + j, o] = w_key[o, j, di, dj]
        wk2[0:64, t, 0:64] = base
        wk2[64:128, t, 64:128] = base

    # e1: split x/k parts with halves stacking
    w1px = np.zeros((64, 32), np.float32)
    w1pk = np.zeros((64, 32), np.float32)
    for r in range(64):
        qx = 2 * r       # x channel r in qk interleave
        qk_ = 2 * r + 1  # k channel r
        if qx < 64:
            w1px[r, 0:16] = w_e1[0:16, qx, 0, 0]
        else:
            w1px[r, 16:32] = w_e1[16:32, qx - 64, 0, 0]
        if qk_ < 64:
            w1pk[r, 0:16] = w_e1[0:16, qk_, 0, 0]
        else:
            w1pk[r, 16:32] = w_e1[16:32, qk_ - 64, 0, 0]
    w1px2 = np.zeros((128, 64), np.float32)
    w1px2[0:64, 0:32] = w1px
    w1px2[64:128, 32:64] = w1px
    w1pk2 = np.zeros((128, 64), np.float32)
    w1pk2[0:64, 0:32] = w1pk
    w1pk2[64:128, 32:64] = w1pk

    # e2 (per half): [32, 72], duplicated to rows 32:64 for the
    # bottom-half matmul (fmap and weights must share a base partition)
    w2p1 = np.zeros((32, 72), np.float32)
    for j in range(32):
        if j < 16:
            w2p1[j, 0:36] = w_e2[0:36, j, 0, 0]
        else:
            w2p1[j, 36:72] = w_e2[36:72, j - 16, 0, 0]
    w2p = np.vstack([w2p1, w2p1])

    # c1 block-diag
    wc1 = np.zeros((64, 64), np.float32)
    for o in range(64):
        if o < 32:
            wc1[0:32, o] = w_c1[o, :, 0, 0]
        else:
            wc1[32:64, o] = w_c1[o, :, 0, 0]
    wc2 = np.zeros((128, 128), np.float32)
    wc2[0:64, 0:64] = wc1
    wc2[64:128, 64:128] = wc1

    g18 = np.zeros((72, 8), np.float32)
    for r in range(72):
        g18[r, r // 9] = 1.0 / 18.0
    b72m = np.zeros((8, 72), np.float32)
    for r in range(72):
        b72m[r // 9, r] = 1.0

    wdu1 = (w_du1[:, :, 0, 0].T / float(NPIX)).astype(np.float32)  # [64, 4]
    wdu12 = np.vstack([wdu1, wdu1])                                # [128, 4]
    wdu2 = w_du2[:, :, 0, 0].T.astype(np.float32)                  # [4, 64]
    wdu22 = np.hstack([wdu2, wdu2])                                # [4, 128]

    return {
        "wk2": wk2.astype(bf),
        "w1px2": w1px2.astype(bf),
        "w1pk2": w1pk2.astype(bf),
        "w2p": w2p.astype(bf),
        "wc2": wc2.astype(bf),
        "b_e2": b_e2.reshape(72, 1).astype(np.float32),
        "gn_w": gn_w.reshape(72, 1).astype(np.float32),
        "gn_b": gn_b.reshape(72, 1).astype(np.float32),
        "g18": g18.astype(bf),
        "b72m": b72m.astype(bf),
        "wdu12": wdu12.astype(bf),
        "wdu22": wdu22.astype(bf),
        "i128": np.eye(128, dtype=np.float32).astype(bf),
        "b128": _b128().astype(bf),
        "gnwi": _gn_per_tap(gn_w),
        "gnbi": _gn_per_tap(gn_b),
    }


def emulate(x, wm):
    """Numpy emulation of the on-core dataflow (f32; validates index maps)."""
    def half_stack(a):  # [64, 128, 128] -> [128, 64, 128]
        return np.concatenate([a[:, 0:64], a[:, 64:128]], axis=0)

    xs = half_stack(x)  # [128, 64, 128]
    # padded X2
    X2 = np.zeros((128, PH2, PW), np.float32)
    X2[:, 1:65, 2:130] = xs
    X2[0:64, 65, 2:130] = x[:, 64]
    X2[64:128, 0, 2:130] = x[:, 63]

    def tapv(buf, di, dj):  # full-image tap view [128, 64, 128]
        return buf[:, di:di + 64, 1 + dj:1 + dj + 128]

    # key conv
    pk = np.zeros((128, 64, 128), np.float32)
    for t in range(9):
        di, dj = t // 3, t % 3
        pk += np.einsum('io,ihw->ohw', wm["wk2"][:, t, :].astype(np.float32),
                        tapv(X2, di, dj))
    K2 = np.maximum(pk, 0)
    # e1
    p1 = (np.einsum('io,ihw->ohw', wm["w1px2"].astype(np.float32), X2[:, 1:65, 2:130])
          + np.einsum('io,ihw->ohw', wm["w1pk2"].astype(np.float32), K2))
    W1c = np.maximum(p1, 0)  # [64, 64, 128]
    # e2 per half + bias
    w2p = wm["w2p"][0:32].astype(np.float32)
    be2 = wm["b_e2"].astype(np.float32)
    W72T = np.einsum('io,ihw->ohw', w2p, W1c[0:32]) + be2[:, None]
    W72B = np.einsum('io,ihw->ohw', w2p, W1c[32:64]) + be2[:, None]
    # GN over both halves
    cat = np.stack([W72T, W72B], axis=1).reshape(8, 18, 64, 128)
    m = cat.mean(axis=(1, 2, 3), keepdims=True)
    v = cat.var(axis=(1, 2, 3), keepdims=True)
    a = (wm["gn_w"].astype(np.float32).reshape(8, 9, 1, 1, 1)
         / np.sqrt(v[:, None, 0] + EPS)[..., None].transpose(0, 1, 2, 3, 4)[:, :, 0:1])
    # simpler: compute rstd per group then per row
    rstd = 1.0 / np.sqrt(v.reshape(8) + EPS)
    mg = m.reshape(8)
    gw = wm["gn_w"].astype(np.float32).reshape(72)
    gb = wm["gn_b"].astype(np.float32).reshape(72)
    alpha = gw * rstd[np.arange(72) // 9]
    beta = gb - mg[np.arange(72) // 9] * alpha
    W72T = W72T * alpha[:, None, None] + beta[:, None, None]
    W72B = W72B * alpha[:, None, None] + beta[:, None, None]
    # c1
    pc = np.einsum('io,ihw->ohw', wm["wc2"].astype(np.float32), X2[:, 1:65, 2:130])
    XV2 = np.zeros((128, PH2, PW), np.float32)
    XV2[:, 1:65, 2:130] = pc
    XV2[0:64, 65, 2:130] = pc[64:128, 0]
    XV2[64:128, 0, 2:130] = pc[0:64, 63]
    # local conv
    OUT2 = np.zeros((128, 64, 128), np.float32)
    gidx = np.arange(128) // 8 * 9  # base row per channel (mod 72 within half)
    for t in range(9):
        di, dj = t // 3, t % 3
        xsv = tapv(XV2, di, dj)
        wbT = W72T[(np.arange(64) // 8) * 9 + t]
        wbB = W72B[(np.arange(64) // 8) * 9 + t]
        wb = np.concatenate([wbT, wbB], axis=0)
        OUT2 += xsv * wb
    # attention
    ysum = OUT2.sum(axis=(1, 2))
    y = ysum @ (wm["wdu12"].astype(np.float32))  # includes both halves + 1/NPIX
    y = np.maximum(y, 0)
    y = y @ wm["wdu22"].astype(np.float32)
    y = 1.0 / (1.0 + np.exp(-y))
    OUT2 = OUT2 * y[:, None, None]
    out = np.concatenate([OUT2[0:64], OUT2[64:128]], axis=1)
    return out


_PROGRAM_CACHE = {}


def _get_program():
    if "nc" not in _PROGRAM_CACHE:
        _PROGRAM_CACHE["nc"] = build_program()
    return _PROGRAM_CACHE["nc"]


def run_on_cores(inputs, trace=False):
    nc = _get_program()
    x = np.asarray(inputs["x"], np.float32)
    wmaps = prep_weights(
        np.asarray(inputs["w_key"], np.float32),
        np.asarray(inputs["w_e1"], np.float32),
        np.asarray(inputs["w_e2"], np.float32),
        np.asarray(inputs["b_e2"], np.float32),
        np.asarray(inputs["gn_w"], np.float32),
        np.asarray(inputs["gn_b"], np.float32),
        np.asarray(inputs["w_c1"], np.float32),
        np.asarray(inputs["w_du1"], np.float32),
        np.asarray(inputs["w_du2"], np.float32),
    )
    in_maps = []
    for b in range(8):
        m = {"x_shard": np.ascontiguousarray(x[b])}
        m.update(wmaps)
        in_maps.append(m)
    res = run_bass_kernel_spmd(nc, in_maps, core_ids=list(range(8)), trace=trace)
    import ml_dtypes
    outs = []
    for b in range(8):
        raw = np.ascontiguousarray(np.asarray(res.results[b]["out"], np.float32))
        bf = raw.view(ml_dtypes.bfloat16).reshape(C, H, W)
        outs.append(bf.astype(np.float32))
    out = np.stack(outs, axis=0)
    return out, res


def kernel(**inputs) -> np.ndarray:
    out, _ = run_on_cores(inputs, trace=False)
    return out.astype(np.float32)


# revision 4
# speedup vs baseline: 1.1107x; 1.0243x over previous
"""Trainium2 Bass kernel v2 for nn_CALayer — halves-stacked layout.

Data parallel over batch B=8 across 8 cores; within a core the image's
two row-halves (rows 0-63 / 64-127) are stacked on SBUF partitions
0-63 / 64-127.  All 64-channel convs become 128-wide block-diagonal
matmuls (half the PE passes of v1), and all element-wise work runs at
full 128-lane DVE width.  The per-pixel kernel broadcast (group row ->
8 channels) runs on the DMA engines via stride-0 access patterns
instead of PE matmuls.
"""

import numpy as np

import concourse.bass as bass
import concourse.tile as tile
from concourse import mybir
from concourse.bass_utils import run_bass_kernel_spmd

F32 = mybir.dt.float32
BF16 = mybir.dt.bfloat16

H = 128
W = 128
C = 64
HH = 64          # rows per half
NPIX = H * W
NPIXH = HH * W   # 8192 pixels per half
PH2 = 66         # padded rows per half (1 + 64 + 1)
PW = 132         # padded width
CH = 512         # psum chunk columns (4 rows per half)
NCH = NPIXH // CH    # 16 chunks
RPC = CH // W        # 4 rows per chunk
BQ = 2048        # big chunk for stage C / loads (16 rows per half)
NBQ = NPIXH // BQ    # 4
RBQ = BQ // W        # 16
EPS = 1e-5

AF = mybir.ActivationFunctionType
ALU = mybir.AluOpType


def _split_big_waits(nc, max_waits=1):
    """walrus CTRL codegen accepts only one sem wait per instruction; move
    extra waits onto Drain instructions inserted just before."""
    from concourse import mybir as _mybir
    n_fixed = 0
    for fn in nc.m.functions:
        for bb in fn.blocks:
            insts = bb.instructions
            i = 0
            while i < len(insts):
                inst = insts[i]
                si = inst.sync_info
                if si is not None and si.on_wait and len(si.on_wait) > max_waits:
                    waits = list(si.on_wait)
                    keep = waits[-max_waits:]
                    extra = waits[:-max_waits]
                    new_insts = []
                    for j in range(0, len(extra), max_waits):
                        chunk = extra[j : j + max_waits]
                        d = _mybir.InstDrain(
                            name=f"{inst.name}-waitsplit{j}", ins=[], outs=[]
                        )
                        d.engine = inst.engine
                        d.sync_info = _mybir.SyncInfo(on_wait=chunk, on_update=[])
                        new_insts.append(d)
                    si.on_wait = keep
                    inst.sync_info = si
                    for k, d in enumerate(new_insts):
                        insts.insert(i + k, d)
                    i += len(new_insts)
                    n_fixed += 1
                i += 1
    return n_fixed



def build_program():
    nc = bass.Bass("TRN2", target_bir_lowering=False, debug=False)

    def din(name, shape, dt=F32):
        return nc.dram_tensor(name, shape, dt, kind="ExternalInput").ap()

    x_in = din("x_shard", [C, H, W])
    wk2 = din("wk2", [128, 9, 128], BF16)     # key conv per-tap block-diag lhsT
    w1px2 = din("w1px2", [128, 64], BF16)     # e1 x-part lhsT (both halves)
    w1pk2 = din("w1pk2", [128, 64], BF16)     # e1 k-part lhsT
    w2p = din("w2p", [64, 72], BF16)          # e2 lhsT (rows 32:64 duplicate)
    wc2 = din("wc2", [128, 128], BF16)        # c1 block-diag lhsT
    be2 = din("b_e2", [72, 1])
    gnw = din("gn_w", [72, 1])
    gnb = din("gn_b", [72, 1])
    g18 = din("g18", [72, 8], BF16)           # group-mean matrix (1/18)
    b72m = din("b72m", [8, 72], BF16)         # group->row broadcast matrix
    wdu12 = din("wdu12", [128, 4], BF16)      # attention fc1 (1/NPIX folded)
    wdu22 = din("wdu22", [4, 128], BF16)      # attention fc2
    i128 = din("i128", [128, 128], BF16)      # identity (PE tap-sum)
    b128 = din("b128", [8, 128], BF16)        # group->channel broadcast
    gnwi = din("gnwi", [128, 9])              # gn_w per (channel, tap)
    gnbi = din("gnbi", [128, 9])              # gn_b per (channel, tap)
    out_d = nc.dram_tensor("out", [C, H, W // 2], F32, kind="ExternalOutput").ap()

    with tile.TileContext(nc) as tc:
        _build(tc, x_in, wk2, w1px2, w1pk2, w2p, wc2, be2, gnw, gnb,
               g18, b72m, wdu12, wdu22, i128, b128, gnwi, gnbi, out_d)

    _split_big_waits(nc)
    return nc


def _build(tc, x_in, wk2, w1px2, w1pk2, w2p, wc2, be2, gnw, gnb,
           g18, b72m, wdu12, wdu22, i128, b128, gnwi, gnbi, out_d):
    nc = tc.nc
    from contextlib import ExitStack

    ctx = ExitStack()
    with ctx:
        big = ctx.enter_context(tc.tile_pool(name="big", bufs=1))
        weights = ctx.enter_context(tc.tile_pool(name="weights", bufs=1))
        small = ctx.enter_context(tc.tile_pool(name="small", bufs=1))
        wbp = ctx.enter_context(tc.tile_pool(name="wbp", bufs=20))
        prp = ctx.enter_context(tc.tile_pool(name="prp", bufs=3))
        w1p_pool = ctx.enter_context(tc.tile_pool(name="w1c", bufs=3))
        otp = ctx.enter_context(tc.tile_pool(name="otp", bufs=3))
        # PSUM: kp(2) + e1(2) + e2T(1) + e2B(1) + small(2) = 8 banks;
        # the stage A pools are released before stage C (identity-sum banks)
        psm = tc.alloc_tile_pool(name="psm", bufs=2, space="PSUM")
        pk = tc.alloc_tile_pool(name="pk", bufs=2, space="PSUM")
        pe1 = tc.alloc_tile_pool(name="pe1", bufs=2, space="PSUM")
        pe2t = tc.alloc_tile_pool(name="pe2t", bufs=1, space="PSUM")
        pe2b = tc.alloc_tile_pool(name="pe2b", bufs=1, space="PSUM")

        # ---- resident buffers ----
        X2 = big.tile([128, PH2, PW], BF16)    # x halves, padded
        XV2 = big.tile([128, PH2, PW], BF16)   # c1 out halves, padded
        W72T = big.tile([72, NPIXH], BF16)     # e2+bias top half
        W72B = big.tile([72, NPIXH], BF16)     # e2+bias bottom half
        OUT2 = big.tile([128, NPIXH], BF16)    # local conv accumulator

        # ---- weights ----
        WK2 = weights.tile([128, 9, 128], BF16)
        nc.sync.dma_start(WK2[:], wk2[:])
        W1PX = weights.tile([128, 64], BF16)
        nc.sync.dma_start(W1PX[:], w1px2[:])
        W1PK = weights.tile([128, 64], BF16)
        nc.sync.dma_start(W1PK[:], w1pk2[:])
        W2P = weights.tile([64, 72], BF16)
        nc.sync.dma_start(W2P[:], w2p[:])
        WC2 = weights.tile([128, 128], BF16)
        nc.sync.dma_start(WC2[:], wc2[:])
        BE2 = weights.tile([72, 1], F32)
        nc.sync.dma_start(BE2[:], be2[:])
        GNW = weights.tile([72, 1], F32)
        nc.sync.dma_start(GNW[:], gnw[:])
        GNB = weights.tile([72, 1], F32)
        nc.sync.dma_start(GNB[:], gnb[:])
        G18 = weights.tile([72, 8], BF16)
        nc.sync.dma_start(G18[:], g18[:])
        B72 = weights.tile([8, 72], BF16)
        nc.sync.dma_start(B72[:], b72m[:])
        WDU1 = weights.tile([128, 4], BF16)
        nc.sync.dma_start(WDU1[:], wdu12[:])
        WDU2 = weights.tile([4, 128], BF16)
        nc.sync.dma_start(WDU2[:], wdu22[:])
        I128 = weights.tile([128, 128], BF16)
        nc.sync.dma_start(I128[:], i128[:])
        B128 = weights.tile([8, 128], BF16)
        nc.sync.dma_start(B128[:], b128[:])
        GNWI = weights.tile([128, 9], F32)
        nc.sync.dma_start(GNWI[:], gnwi[:])
        GNBI = weights.tile([128, 9], F32)
        nc.sync.dma_start(GNBI[:], gnbi[:])

        # ---- zero pads (only the pad regions) ----
        for buf in (X2, XV2):
            nc.gpsimd.memset(buf[:, :, 0:2], 0.0)
            nc.gpsimd.memset(buf[:, :, 130:132], 0.0)
            nc.gpsimd.memset(buf[0:64, 0:1, :], 0.0)
            nc.gpsimd.memset(buf[64:128, 65:66, :], 0.0)

        # ---- load x via gpsimd casting DMA (f32 HBM -> bf16 padded SBUF);
        # halo rows first (needed by the first chunks) ----
        nc.gpsimd.dma_start(X2[0:64, 65:66, 2:130], x_in[:, 64:65, :])
        nc.gpsimd.dma_start(X2[64:128, 0:1, 2:130], x_in[:, 63:64, :])
        for lq in range(NBQ):
            r0 = lq * RBQ
            nc.gpsimd.dma_start(X2[0:64, 1 + r0:1 + r0 + RBQ, 2:130],
                                x_in[:, r0:r0 + RBQ, :])
            nc.gpsimd.dma_start(X2[64:128, 1 + r0:1 + r0 + RBQ, 2:130],
                                x_in[:, 64 + r0:64 + r0 + RBQ, :])

        taps = [(di, dj) for di in range(3) for dj in range(3)]
        wbs = {}
        statsT = small.tile([72, NCH, 6], F32)
        statsB = small.tile([72, NCH, 6], F32)

        def img(buf, h0, nrows=RPC):
            return buf[:, 1 + h0:1 + h0 + nrows, 2:2 + W]

        def tapv(buf, h0, di, dj, nrows=RPC):
            return buf[:, h0 + di:h0 + di + nrows, 1 + dj:1 + dj + W]

        # ======== stage A: per-chunk convs (pairs of chunks) ========
        for qp in range(NCH // 2):
            qa, qb = 2 * qp, 2 * qp + 1
            pks = {}
            # key conv: tap-outer over the chunk pair
            for t in range(9):
                di, dj = taps[t]
                for q in (qa, qb):
                    if t == 0:
                        pks[q] = pk.tile([128, RPC, W], F32, tag="kp", name="pkq")
                    nc.tensor.matmul(
                        pks[q][:], WK2[:, t, :], tapv(X2, q * RPC, di, dj),
                        start=(t == 0), stop=(t == 8),
                    )
            k2s = {}
            for q in (qa, qb):
                k2s[q] = w1p_pool.tile([128, CH], BF16, tag="k2", name="k2q")
                nc.scalar.activation(
                    k2s[q][:].rearrange("p (a b) -> p a b", a=RPC),
                    pks[q][:], AF.Relu)
            # e1: two accumulating 64-contraction matmuls
            p1s = {}
            for q in (qa, qb):
                p1s[q] = pe1.tile([64, CH], F32, tag="e1", name="p1q")
                nc.tensor.matmul(
                    p1s[q][:],
                    W1PX[:], img(X2, q * RPC),
                    start=True, stop=False)
            for q in (qa, qb):
                nc.tensor.matmul(
                    p1s[q][:], W1PK[:], k2s[q][:], start=False, stop=True)
            w1cs = {}
            for q in (qa, qb):
                w1cs[q] = w1p_pool.tile([64, CH], BF16, tag="w1c", name="w1cq")
                nc.scalar.activation(w1cs[q][:], p1s[q][:], AF.Relu)
            # e2 per half (+bias on the ACT copy out of psum)
            for q in (qa, qb):
                qs = slice(q * CH, (q + 1) * CH)
                pt = pe2t.tile([72, CH], F32, tag="e2t")
                nc.tensor.matmul(pt[:], W2P[0:32, :], w1cs[q][0:32, :],
                                 start=True, stop=True)
                nc.scalar.activation(W72T[:, qs], pt[:], AF.Identity,
                                     bias=BE2[:])
                pb = pe2b.tile([72, CH], F32, tag="e2b")
                nc.tensor.matmul(pb[:], W2P[32:64, :], w1cs[q][32:64, :],
                                 start=True, stop=True)
                nc.scalar.activation(W72B[:, qs], pb[:], AF.Identity,
                                     bias=BE2[:])
            # c1 (reuses the key psum ring)
            for q in (qa, qb):
                pc = pk.tile([128, RPC, W], F32, tag="kp")
                nc.tensor.matmul(pc[:], WC2[:], img(X2, q * RPC),
                                 start=True, stop=True)
                nc.vector.tensor_copy(img(XV2, q * RPC), pc[:])
            # GN statistics (bn_stats free dim is capped at 512)
            for q in (qa, qb):
                qs = slice(q * CH, (q + 1) * CH)
                nc.vector.bn_stats(out=statsT[:, q, :], in_=W72T[:, qs])
                nc.vector.bn_stats(out=statsB[:, q, :], in_=W72B[:, qs])
            if qp == 0:
                # bottom half's upper halo = top half's chunk-0 output... no:
                # XV2[64:128, 0] (bottom halo) = xv image row 63 -> top half
                # local row 63 = chunk 15 (NOT ready yet).
                # XV2[0:64, 65] (top halo) = xv image row 64 -> bottom half
                # local row 0 = chunk 0 (ready now).
                nc.sync.dma_start(XV2[0:64, 65:66, 2:130], XV2[64:128, 1:2, 2:130])
            if qp == NCH // 2 - 1:
                # on the scalar queue: the sync queue may be stalled on wb
                # ring slots whose release needs stage C (which reads this)
                nc.scalar.dma_start(XV2[64:128, 0:1, 2:130], XV2[0:64, 64:65, 2:130])
            if qp % 2 == 1 and qp // 2 < 3:
                # raw kernel broadcasts for the completed big chunk (sync
                # hwdge; GN scale/bias is applied post-broadcast).  Only the
                # first two big chunks are prefetched raw; the last two are
                # broadcast after GN is applied directly on W72.
                bqd = qp // 2
                bs = slice(bqd * BQ, (bqd + 1) * BQ)
                for t in range(9):
                    wb = wbp.tile([128, BQ], BF16, tag="wb", name="wbt")
                    sT = W72T[:][t:72:9, bs].unsqueeze(1).broadcast_to([8, 8, BQ])
                    sB = W72B[:][t:72:9, bs].unsqueeze(1).broadcast_to([8, 8, BQ])
                    nc.sync.dma_start(wb[0:64, :], sT)
                    nc.sync.dma_start(wb[64:128, :], sB)
                    wbs[(bqd, t)] = wb

        # ======== stage B: GroupNorm scale/bias ========
        mvT = small.tile([72, 2], F32)
        nc.vector.bn_aggr(out=mvT[:], in_=statsT[:])
        mvB = small.tile([72, 2], F32)
        nc.vector.bn_aggr(out=mvB[:], in_=statsB[:])
        packT = small.tile([72, 2], BF16)
        packB = small.tile([72, 2], BF16)
        for mv, pack in ((mvT, packT), (mvB, packB)):
            rowq = small.tile([72, 1], F32, tag="rowq")
            nc.vector.tensor_mul(rowq[:], mv[:, 0:1], mv[:, 0:1])
            nc.vector.tensor_add(rowq[:], rowq[:], mv[:, 1:2])
            nc.vector.tensor_copy(pack[:, 0:1], mv[:, 0:1])
            nc.vector.tensor_copy(pack[:, 1:2], rowq[:])
        pg = psm.tile([128, 2], F32, tag="sp")
        nc.tensor.matmul(pg[0:8, :], G18[:], packT[:], start=True, stop=False)
        nc.tensor.matmul(pg[0:8, :], G18[:], packB[:], start=False, stop=True)
        gm = small.tile([8, 2], F32)
        nc.vector.tensor_copy(gm[:], pg[0:8, :])
        msq = small.tile([8, 1], F32)
        nc.vector.tensor_mul(msq[:], gm[:, 0:1], gm[:, 0:1])
        v8 = small.tile([8, 1], F32)
        nc.vector.tensor_tensor(out=v8[:], in0=gm[:, 1:2], in1=msq[:],
                                op=ALU.subtract)
        eps8 = small.tile([8, 1], F32)
        nc.vector.memset(eps8[:], EPS)
        sd8 = small.tile([8, 1], F32)
        nc.scalar.activation(sd8[:], v8[:], AF.Sqrt, bias=eps8[:])
        rstd8 = small.tile([8, 2], F32)
        nc.vector.reciprocal(rstd8[:, 0:1], sd8[:])
        nc.vector.tensor_copy(rstd8[:, 1:2], gm[:, 0:1])
        rstd8b = small.tile([8, 2], BF16)
        nc.vector.tensor_copy(rstd8b[:], rstd8[:])
        # broadcast (rstd, m) to channels (tap-independent), then build the
        # per-(channel, tap) scale/bias tables with wide DVE ops
        p128 = psm.tile([128, 2], F32, tag="sp")
        nc.tensor.matmul(p128[:], B128[:], rstd8b[:], start=True, stop=True)
        r128 = small.tile([128, 2], F32)
        nc.vector.tensor_copy(r128[:], p128[:])
        ABTa = small.tile([128, 9], F32)
        nc.vector.tensor_scalar(out=ABTa[:], in0=GNWI[:], scalar1=r128[:, 0:1],
                                scalar2=None, op0=ALU.mult)
        ABTb = small.tile([128, 9], F32)
        nc.vector.tensor_scalar(out=ABTb[:], in0=ABTa[:], scalar1=r128[:, 1:2],
                                scalar2=None, op0=ALU.mult)
        nc.vector.tensor_tensor(out=ABTb[:], in0=GNBI[:], in1=ABTb[:],
                                op=ALU.subtract)
        # per-row scale/bias for the in-place W72 GN-apply (bq2/3 source)
        p72 = psm.tile([128, 2], F32, tag="sp")
        nc.tensor.matmul(p72[0:72, :], B72[:], rstd8b[:], start=True, stop=True)
        rs72 = small.tile([72, 2], F32)
        nc.vector.tensor_copy(rs72[:], p72[0:72, :])
        a72 = small.tile([72, 1], F32)
        nc.vector.tensor_mul(a72[:], rs72[:, 0:1], GNW[:])
        b72 = small.tile([72, 1], F32)
        nc.vector.tensor_mul(b72[:], rs72[:, 1:2], a72[:])
        nc.vector.tensor_tensor(out=b72[:], in0=GNB[:], in1=b72[:],
                                op=ALU.subtract)

        pe2b.release()
        pe2t.release()
        pe1.release()
        pk.release()
        psm.release()
        pcs = tc.alloc_tile_pool(name="pcs", bufs=8, space="PSUM")

        # ======== stage C: normalize prefetched kernels + local conv ========
        # products on DVE; the tap-sum runs on the (idle) PE as accumulating
        # identity matmuls into PSUM (f32), drained by DVE copies w/ row-sums
        ys16 = small.tile([128, NCH], F32)
        for bq in range(NBQ):
            r0 = bq * RBQ
            banks = {}
            pa = None
            for t in range(9):
                di, dj = taps[t]
                wb = wbs.pop((bq, t))
                if bq == 3:
                    pass  # already normalized at the source
                elif bq == 1 or t in (0, 1, 2, 3, 7):
                    # bq1 stays off gpsimd: the gp queue carries the bq3
                    # broadcast issues which stall on wb ring slots that
                    # bq1's products release
                    nc.scalar.activation(wb[:], wb[:], AF.Identity,
                                         bias=ABTb[:, t:t + 1], scale=ABTa[:, t:t + 1])
                else:
                    nc.gpsimd.tensor_scalar(
                        out=wb[:], in0=wb[:], scalar1=ABTa[:, t:t + 1],
                        scalar2=ABTb[:, t:t + 1], op0=ALU.mult, op1=ALU.add)
                wbv = wb[:].rearrange("p (a b) -> p a b", a=RBQ)
                xs = tapv(XV2, r0, di, dj, nrows=RBQ)
                if t < 6:
                    # PE-summed taps
                    p = prp.tile([128, BQ], BF16, tag="p")
                    pv = p[:].rearrange("p (a b) -> p a b", a=RBQ)
                    nc.vector.tensor_mul(pv, xs, wbv)
                    for c in range(4):
                        if t == 0:
                            banks[c] = pcs.tile([128, CH], F32, tag="cb",
                                                name="cbank")
                        nc.tensor.matmul(
                            banks[c][:], I128[:], p[:, c * CH:(c + 1) * CH],
                            start=(t == 0), stop=(t == 5))
                elif t == 6:
                    pa = prp.tile([128, BQ], BF16, tag="pa", name="pacc", bufs=2)
                    nc.vector.tensor_mul(
                        pa[:].rearrange("p (a b) -> p a b", a=RBQ), xs, wbv)
                else:
                    p = prp.tile([128, BQ], BF16, tag="p")
                    pv = p[:].rearrange("p (a b) -> p a b", a=RBQ)
                    nc.vector.tensor_mul(pv, xs, wbv)
                    nc.vector.tensor_add(pa[:], pa[:], p[:])
            for c in range(4):
                q = bq * 4 + c
                nc.vector.tensor_tensor(
                    out=OUT2[:, q * CH:(q + 1) * CH], in0=banks[c][:],
                    in1=pa[:, c * CH:(c + 1) * CH], op=ALU.add)
                nc.vector.tensor_reduce(
                    ys16[:, q:q + 1], OUT2[:, q * CH:(q + 1) * CH],
                    axis=mybir.AxisListType.X, op=ALU.add)
            if bq == 0:
                # GN-apply in place on the second half of W72, then broadcast
                # the already-normalized kernels for big chunks 2-3 (overlaps
                # with big-chunk-1 compute)
                half = slice(3 * BQ, 4 * BQ)
                nc.scalar.activation(W72T[:, half], W72T[:, half], AF.Identity,
                                     bias=b72[:], scale=a72[:])
                nc.gpsimd.tensor_scalar(out=W72B[:, half], in0=W72B[:, half],
                                        scalar1=a72[:], scalar2=b72[:],
                                        op0=ALU.mult, op1=ALU.add)
                for bqd in (3,):
                    bs2 = slice(bqd * BQ, (bqd + 1) * BQ)
                    for t in range(9):
                        wb = wbp.tile([128, BQ], BF16, tag="wb", name="wbt")
                        sT = W72T[:][t:72:9, bs2].unsqueeze(1).broadcast_to([8, 8, BQ])
                        sB = W72B[:][t:72:9, bs2].unsqueeze(1).broadcast_to([8, 8, BQ])
                        nc.sync.dma_start(wb[0:64, :], sT)
                        nc.sync.dma_start(wb[64:128, :], sB)
                        wbs[(bqd, t)] = wb

        pcs.release()
        psm2 = tc.alloc_tile_pool(name="psm2", bufs=2, space="PSUM")

        # ======== stage D: channel attention + store ========
        ysum = small.tile([128, 1], F32)
        nc.vector.tensor_reduce(ysum[:], ys16[:], axis=mybir.AxisListType.X,
                                op=ALU.add)
        ysb = small.tile([128, 1], BF16)
        nc.vector.tensor_copy(ysb[:], ysum[:])
        pa1 = psm2.tile([128, 2], F32, tag="sp")
        nc.tensor.matmul(pa1[0:4, 0:1], WDU1[:], ysb[:], start=True, stop=True)
        y1 = small.tile([4, 1], BF16)
        nc.scalar.activation(y1[:], pa1[0:4, 0:1], AF.Relu)
        pa2 = psm2.tile([128, 2], F32, tag="sp")
        nc.tensor.matmul(pa2[:, 0:1], WDU2[:], y1[:], start=True, stop=True)
        yatt = small.tile([128, 1], F32)
        nc.scalar.activation(yatt[:], pa2[:, 0:1], AF.Sigmoid)

        for hb in range(NBQ):
            r0 = hb * RBQ
            bs = slice(hb * BQ, (hb + 1) * BQ)
            ot = otp.tile([128, RBQ, W], BF16, tag="ot")
            if hb % 2 == 0:
                nc.scalar.activation(
                    ot[:].rearrange("p a b -> p (a b)"), OUT2[:, bs],
                    AF.Identity, scale=yatt[:])
            else:
                nc.vector.tensor_scalar_mul(
                    ot[:].rearrange("p a b -> p (a b)"), OUT2[:, bs], yatt[:])
            otf = ot[:].bitcast(F32)
            nc.sync.dma_start(out_d[:, r0:r0 + RBQ, :], otf[0:64])
            nc.scalar.dma_start(out_d[:, 64 + r0:64 + r0 + RBQ, :], otf[64:128])
        psm2.release()


def _b128():
    m = np.zeros((8, 128), np.float32)
    for c in range(128):
        m[(c % 64) // 8, c] = 1.0
    return m


def _gn_per_tap(v):
    v = np.asarray(v, np.float32).reshape(72)
    out = np.zeros((128, 9), np.float32)
    for c in range(128):
        g = (c % 64) // 8
        for t in range(9):
            out[c, t] = v[g * 9 + t]
    return out


def prep_weights(w_key, w_e1, w_e2, b_e2, gn_w, gn_b, w_c1, w_du1, w_du2):
    import ml_dtypes

    bf = ml_dtypes.bfloat16
    # key conv: per tap block-diag [9, 128, 128]
    wk2 = np.zeros((128, 9, 128), np.float32)
    for t in range(9):
        di, dj = t // 3, t % 3
        base = np.zeros((64, 64), np.float32)
        for o in range(64):
            g = o // 8
            for j in range(8):
                base[g * 8 + j, o] = w_key[o, j, di, dj]
        wk2[0:64, t, 0:64] = base
        wk2[64:128, t, 64:128] = base

    # e1: split x/k parts with halves stacking
    w1px = np.zeros((64, 32), np.float32)
    w1pk = np.zeros((64, 32), np.float32)
    for r in range(64):
        qx = 2 * r       # x channel r in qk interleave
        qk_ = 2 * r + 1  # k channel r
        if qx < 64:
            w1px[r, 0:16] = w_e1[0:16, qx, 0, 0]
        else:
            w1px[r, 16:32] = w_e1[16:32, qx - 64, 0, 0]
        if qk_ < 64:
            w1pk[r, 0:16] = w_e1[0:16, qk_, 0, 0]
        else:
            w1pk[r, 16:32] = w_e1[16:32, qk_ - 64, 0, 0]
    w1px2 = np.zeros((128, 64), np.float32)
    w1px2[0:64, 0:32] = w1px
    w1px2[64:128, 32:64] = w1px
    w1pk2 = np.zeros((128, 64), np.float32)
    w1pk2[0:64, 0:32] = w1pk
    w1pk2[64:128, 32:64] = w1pk

    # e2 (per half): [32, 72], duplicated to rows 32:64 for the
    # bottom-half matmul (fmap and weights must share a base partition)
    w2p1 = np.zeros((32, 72), np.float32)
    for j in range(32):
        if j < 16:
            w2p1[j, 0:36] = w_e2[0:36, j, 0, 0]
        else:
            w2p1[j, 36:72] = w_e2[36:72, j - 16, 0, 0]
    w2p = np.vstack([w2p1, w2p1])

    # c1 block-diag
    wc1 = np.zeros((64, 64), np.float32)
    for o in range(64):
        if o < 32:
            wc1[0:32, o] = w_c1[o, :, 0, 0]
        else:
            wc1[32:64, o] = w_c1[o, :, 0, 0]
    wc2 = np.zeros((128, 128), np.float32)
    wc2[0:64, 0:64] = wc1
    wc2[64:128, 64:128] = wc1

    g18 = np.zeros((72, 8), np.float32)
    for r in range(72):
        g18[r, r // 9] = 1.0 / 18.0
    b72m = np.zeros((8, 72), np.float32)
    for r in range(72):
        b72m[r // 9, r] = 1.0

    wdu1 = (w_du1[:, :, 0, 0].T / float(NPIX)).astype(np.float32)  # [64, 4]
    wdu12 = np.vstack([wdu1, wdu1])                                # [128, 4]
    wdu2 = w_du2[:, :, 0, 0].T.astype(np.float32)                  # [4, 64]
    wdu22 = np.hstack([wdu2, wdu2])                                # [4, 128]

    return {
        "wk2": wk2.astype(bf),
        "w1px2": w1px2.astype(bf),
        "w1pk2": w1pk2.astype(bf),
        "w2p": w2p.astype(bf),
        "wc2": wc2.astype(bf),
        "b_e2": b_e2.reshape(72, 1).astype(np.float32),
        "gn_w": gn_w.reshape(72, 1).astype(np.float32),
        "gn_b": gn_b.reshape(72, 1).astype(np.float32),
        "g18": g18.astype(bf),
        "b72m": b72m.astype(bf),
        "wdu12": wdu12.astype(bf),
        "wdu22": wdu22.astype(bf),
        "i128": np.eye(128, dtype=np.float32).astype(bf),
        "b128": _b128().astype(bf),
        "gnwi": _gn_per_tap(gn_w),
        "gnbi": _gn_per_tap(gn_b),
    }


def emulate(x, wm):
    """Numpy emulation of the on-core dataflow (f32; validates index maps)."""
    def half_stack(a):  # [64, 128, 128] -> [128, 64, 128]
        return np.concatenate([a[:, 0:64], a[:, 64:128]], axis=0)

    xs = half_stack(x)  # [128, 64, 128]
    # padded X2
    X2 = np.zeros((128, PH2, PW), np.float32)
    X2[:, 1:65, 2:130] = xs
    X2[0:64, 65, 2:130] = x[:, 64]
    X2[64:128, 0, 2:130] = x[:, 63]

    def tapv(buf, di, dj):  # full-image tap view [128, 64, 128]
        return buf[:, di:di + 64, 1 + dj:1 + dj + 128]

    # key conv
    pk = np.zeros((128, 64, 128), np.float32)
    for t in range(9):
        di, dj = t // 3, t % 3
        pk += np.einsum('io,ihw->ohw', wm["wk2"][:, t, :].astype(np.float32),
                        tapv(X2, di, dj))
    K2 = np.maximum(pk, 0)
    # e1
    p1 = (np.einsum('io,ihw->ohw', wm["w1px2"].astype(np.float32), X2[:, 1:65, 2:130])
          + np.einsum('io,ihw->ohw', wm["w1pk2"].astype(np.float32), K2))
    W1c = np.maximum(p1, 0)  # [64, 64, 128]
    # e2 per half + bias
    w2p = wm["w2p"][0:32].astype(np.float32)
    be2 = wm["b_e2"].astype(np.float32)
    W72T = np.einsum('io,ihw->ohw', w2p, W1c[0:32]) + be2[:, None]
    W72B = np.einsum('io,ihw->ohw', w2p, W1c[32:64]) + be2[:, None]
    # GN over both halves
    cat = np.stack([W72T, W72B], axis=1).reshape(8, 18, 64, 128)
    m = cat.mean(axis=(1, 2, 3), keepdims=True)
    v = cat.var(axis=(1, 2, 3), keepdims=True)
    a = (wm["gn_w"].astype(np.float32).reshape(8, 9, 1, 1, 1)
         / np.sqrt(v[:, None, 0] + EPS)[..., None].transpose(0, 1, 2, 3, 4)[:, :, 0:1])
    # simpler: compute rstd per group then per row
    rstd = 1.0 / np.sqrt(v.reshape(8) + EPS)
    mg = m.reshape(8)
    gw = wm["gn_w"].astype(np.float32).reshape(72)
    gb = wm["gn_b"].astype(np.float32).reshape(72)
    alpha = gw * rstd[np.arange(72) // 9]
    beta = gb - mg[np.arange(72) // 9] * alpha
    W72T = W72T * alpha[:, None, None] + beta[:, None, None]
    W72B = W72B * alpha[:, None, None] + beta[:, None, None]
    # c1
    pc = np.einsum('io,ihw->ohw', wm["wc2"].astype(np.float32), X2[:, 1:65, 2:130])
    XV2 = np.zeros((128, PH2, PW), np.float32)
    XV2[:, 1:65, 2:130] = pc
    XV2[0:64, 65, 2:130] = pc[64:128, 0]
    XV2[64:128, 0, 2:130] = pc[0:64, 63]
    # local conv
    OUT2 = np.zeros((128, 64, 128), np.float32)
    gidx = np.arange(128) // 8 * 9  # base row per channel (mod 72 within half)
    for t in range(9):
        di, dj = t // 3, t % 3
        xsv = tapv(XV2, di, dj)
        wbT = W72T[(np.arange(64) // 8) * 9 + t]
        wbB = W72B[(np.arange(64) // 8) * 9 + t]
        wb = np.concatenate([wbT, wbB], axis=0)
        OUT2 += xsv * wb
    # attention
    ysum = OUT2.sum(axis=(1, 2))
    y = ysum @ (wm["wdu12"].astype(np.float32))  # includes both halves + 1/NPIX
    y = np.maximum(y, 0)
    y = y @ wm["wdu22"].astype(np.float32)
    y = 1.0 / (1.0 + np.exp(-y))
    OUT2 = OUT2 * y[:, None, None]
    out = np.concatenate([OUT2[0:64], OUT2[64:128]], axis=1)
    return out


_PROGRAM_CACHE = {}


def _get_program():
    if "nc" not in _PROGRAM_CACHE:
        _PROGRAM_CACHE["nc"] = build_program()
    return _PROGRAM_CACHE["nc"]


def run_on_cores(inputs, trace=False):
    nc = _get_program()
    x = np.asarray(inputs["x"], np.float32)
    wmaps = prep_weights(
        np.asarray(inputs["w_key"], np.float32),
        np.asarray(inputs["w_e1"], np.float32),
        np.asarray(inputs["w_e2"], np.float32),
        np.asarray(inputs["b_e2"], np.float32),
        np.asarray(inputs["gn_w"], np.float32),
        np.asarray(inputs["gn_b"], np.float32),
        np.asarray(inputs["w_c1"], np.float32),
        np.asarray(inputs["w_du1"], np.float32),
        np.asarray(inputs["w_du2"], np.float32),
    )
    in_maps = []
    for b in range(8):
        m = {"x_shard": np.ascontiguousarray(x[b])}
        m.update(wmaps)
        in_maps.append(m)
    res = run_bass_kernel_spmd(nc, in_maps, core_ids=list(range(8)), trace=trace)
    import ml_dtypes
    outs = []
    for b in range(8):
        raw = np.ascontiguousarray(np.asarray(res.results[b]["out"], np.float32))
        bf = raw.view(ml_dtypes.bfloat16).reshape(C, H, W)
        outs.append(bf.astype(np.float32))
    out = np.stack(outs, axis=0)
    return out, res


def kernel(**inputs) -> np.ndarray:
    out, _ = run_on_cores(inputs, trace=False)
    return out.astype(np.float32)


# revision 5
# speedup vs baseline: 1.1325x; 1.0196x over previous
"""Trainium2 Bass kernel v2 for nn_CALayer — halves-stacked layout.

Data parallel over batch B=8 across 8 cores; within a core the image's
two row-halves (rows 0-63 / 64-127) are stacked on SBUF partitions
0-63 / 64-127.  All 64-channel convs become 128-wide block-diagonal
matmuls (half the PE passes of v1), and all element-wise work runs at
full 128-lane DVE width.  The per-pixel kernel broadcast (group row ->
8 channels) runs on the DMA engines via stride-0 access patterns
instead of PE matmuls.
"""

import numpy as np

import concourse.bass as bass
import concourse.tile as tile
from concourse import mybir
from concourse.bass_utils import run_bass_kernel_spmd

F32 = mybir.dt.float32
BF16 = mybir.dt.bfloat16

H = 128
W = 128
C = 64
HH = 64          # rows per half
NPIX = H * W
NPIXH = HH * W   # 8192 pixels per half
PH2 = 66         # padded rows per half (1 + 64 + 1)
PW = 132         # padded width
CH = 512         # psum chunk columns (4 rows per half)
NCH = NPIXH // CH    # 16 chunks
RPC = CH // W        # 4 rows per chunk
BQ = 2048        # big chunk for stage C / loads (16 rows per half)
NBQ = NPIXH // BQ    # 4
RBQ = BQ // W        # 16
EPS = 1e-5

AF = mybir.ActivationFunctionType
ALU = mybir.AluOpType


def _split_big_waits(nc, max_waits=1):
    """walrus CTRL codegen accepts only one sem wait per instruction; move
    extra waits onto Drain instructions inserted just before."""
    from concourse import mybir as _mybir
    n_fixed = 0
    for fn in nc.m.functions:
        for bb in fn.blocks:
            insts = bb.instructions
            i = 0
            while i < len(insts):
                inst = insts[i]
                si = inst.sync_info
                if si is not None and si.on_wait and len(si.on_wait) > max_waits:
                    waits = list(si.on_wait)
                    keep = waits[-max_waits:]
                    extra = waits[:-max_waits]
                    new_insts = []
                    for j in range(0, len(extra), max_waits):
                        chunk = extra[j : j + max_waits]
                        d = _mybir.InstDrain(
                            name=f"{inst.name}-waitsplit{j}", ins=[], outs=[]
                        )
                        d.engine = inst.engine
                        d.sync_info = _mybir.SyncInfo(on_wait=chunk, on_update=[])
                        new_insts.append(d)
                    si.on_wait = keep
                    inst.sync_info = si
                    for k, d in enumerate(new_insts):
                        insts.insert(i + k, d)
                    i += len(new_insts)
                    n_fixed += 1
                i += 1
    return n_fixed



def build_program():
    nc = bass.Bass("TRN2", target_bir_lowering=False, debug=False)

    def din(name, shape, dt=F32):
        return nc.dram_tensor(name, shape, dt, kind="ExternalInput").ap()

    x_in = din("x_shard", [C, H, W])
    wk2 = din("wk2", [128, 9, 128], BF16)     # key conv per-tap block-diag lhsT
    w1px2 = din("w1px2", [128, 64], BF16)     # e1 x-part lhsT (both halves)
    w1pk2 = din("w1pk2", [128, 64], BF16)     # e1 k-part lhsT
    w2p = din("w2p", [64, 72], BF16)          # e2 lhsT (rows 32:64 duplicate)
    wc2 = din("wc2", [128, 128], BF16)        # c1 block-diag lhsT
    be2 = din("b_e2", [72, 1])
    gnw = din("gn_w", [72, 1])
    gnb = din("gn_b", [72, 1])
    g18 = din("g18", [72, 8], BF16)           # group-mean matrix (1/18)
    b72m = din("b72m", [8, 72], BF16)         # group->row broadcast matrix
    wdu12 = din("wdu12", [128, 4], BF16)      # attention fc1 (1/NPIX folded)
    wdu22 = din("wdu22", [4, 128], BF16)      # attention fc2
    i128 = din("i128", [128, 128], BF16)      # identity (PE tap-sum)
    b128 = din("b128", [8, 128], BF16)        # group->channel broadcast
    gnwi = din("gnwi", [128, 9])              # gn_w per (channel, tap)
    gnbi = din("gnbi", [128, 9])              # gn_b per (channel, tap)
    out_d = nc.dram_tensor("out", [C, H, W // 2], F32, kind="ExternalOutput").ap()

    with tile.TileContext(nc) as tc:
        _build(tc, x_in, wk2, w1px2, w1pk2, w2p, wc2, be2, gnw, gnb,
               g18, b72m, wdu12, wdu22, i128, b128, gnwi, gnbi, out_d)

    _split_big_waits(nc)
    return nc


def _build(tc, x_in, wk2, w1px2, w1pk2, w2p, wc2, be2, gnw, gnb,
           g18, b72m, wdu12, wdu22, i128, b128, gnwi, gnbi, out_d):
    nc = tc.nc
    from contextlib import ExitStack

    ctx = ExitStack()
    with ctx:
        big = ctx.enter_context(tc.tile_pool(name="big", bufs=1))
        weights = ctx.enter_context(tc.tile_pool(name="weights", bufs=1))
        small = ctx.enter_context(tc.tile_pool(name="small", bufs=1))
        wbp = ctx.enter_context(tc.tile_pool(name="wbp", bufs=20))
        prp = ctx.enter_context(tc.tile_pool(name="prp", bufs=3))
        w1p_pool = ctx.enter_context(tc.tile_pool(name="w1c", bufs=3))
        otp = ctx.enter_context(tc.tile_pool(name="otp", bufs=3))
        # PSUM: kp(2) + e1(2) + e2T(1) + e2B(1) + small(2) = 8 banks;
        # the stage A pools are released before stage C (identity-sum banks)
        psm = tc.alloc_tile_pool(name="psm", bufs=2, space="PSUM")
        pk = tc.alloc_tile_pool(name="pk", bufs=2, space="PSUM")
        pe1 = tc.alloc_tile_pool(name="pe1", bufs=2, space="PSUM")
        pe2t = tc.alloc_tile_pool(name="pe2t", bufs=1, space="PSUM")
        pe2b = tc.alloc_tile_pool(name="pe2b", bufs=1, space="PSUM")

        # ---- resident buffers ----
        X2 = big.tile([128, PH2, PW], BF16)    # x halves, padded
        XV2 = big.tile([128, PH2, PW], BF16)   # c1 out halves, padded
        W72T = big.tile([72, NPIXH], BF16)     # e2+bias top half
        W72B = big.tile([72, NPIXH], BF16)     # e2+bias bottom half
        OUT2 = big.tile([128, NPIXH], BF16)    # local conv accumulator

        # ---- weights ----
        WK2 = weights.tile([128, 9, 128], BF16)
        nc.sync.dma_start(WK2[:], wk2[:])
        W1PX = weights.tile([128, 64], BF16)
        nc.sync.dma_start(W1PX[:], w1px2[:])
        W1PK = weights.tile([128, 64], BF16)
        nc.sync.dma_start(W1PK[:], w1pk2[:])
        W2P = weights.tile([64, 72], BF16)
        nc.sync.dma_start(W2P[:], w2p[:])
        WC2 = weights.tile([128, 128], BF16)
        nc.sync.dma_start(WC2[:], wc2[:])
        BE2 = weights.tile([72, 1], F32)
        nc.sync.dma_start(BE2[:], be2[:])
        GNW = weights.tile([72, 1], F32)
        nc.sync.dma_start(GNW[:], gnw[:])
        GNB = weights.tile([72, 1], F32)
        nc.sync.dma_start(GNB[:], gnb[:])
        G18 = weights.tile([72, 8], BF16)
        nc.sync.dma_start(G18[:], g18[:])
        B72 = weights.tile([8, 72], BF16)
        nc.sync.dma_start(B72[:], b72m[:])
        WDU1 = weights.tile([128, 4], BF16)
        nc.sync.dma_start(WDU1[:], wdu12[:])
        WDU2 = weights.tile([4, 128], BF16)
        nc.sync.dma_start(WDU2[:], wdu22[:])
        I128 = weights.tile([128, 128], BF16)
        nc.sync.dma_start(I128[:], i128[:])
        B128 = weights.tile([8, 128], BF16)
        nc.sync.dma_start(B128[:], b128[:])
        GNWI = weights.tile([128, 9], F32)
        nc.sync.dma_start(GNWI[:], gnwi[:])
        GNBI = weights.tile([128, 9], F32)
        nc.sync.dma_start(GNBI[:], gnbi[:])

        # ---- zero pads (only the pad regions) ----
        for buf in (X2, XV2):
            nc.gpsimd.memset(buf[:, :, 0:2], 0.0)
            nc.gpsimd.memset(buf[:, :, 130:132], 0.0)
            nc.gpsimd.memset(buf[0:64, 0:1, :], 0.0)
            nc.gpsimd.memset(buf[64:128, 65:66, :], 0.0)

        # ---- load x via gpsimd casting DMA (f32 HBM -> bf16 padded SBUF);
        # halo rows first (needed by the first chunks) ----
        nc.gpsimd.dma_start(X2[64:128, 0:1, 2:130], x_in[:, 63:64, :])
        nc.gpsimd.dma_start(X2[0:64, 65:66, 2:130], x_in[:, 64:65, :])
        # big chunk 0 in 4-row pieces so the first key convs start early
        for pc in range(4):
            r0 = pc * 4
            nc.gpsimd.dma_start(X2[0:64, 1 + r0:5 + r0, 2:130],
                                x_in[:, r0:r0 + 4, :])
            nc.gpsimd.dma_start(X2[64:128, 1 + r0:5 + r0, 2:130],
                                x_in[:, 64 + r0:64 + r0 + 4, :])
        for lq in range(1, NBQ):
            r0 = lq * RBQ
            nc.gpsimd.dma_start(X2[0:64, 1 + r0:1 + r0 + RBQ, 2:130],
                                x_in[:, r0:r0 + RBQ, :])
            nc.gpsimd.dma_start(X2[64:128, 1 + r0:1 + r0 + RBQ, 2:130],
                                x_in[:, 64 + r0:64 + r0 + RBQ, :])

        taps = [(di, dj) for di in range(3) for dj in range(3)]
        wbs = {}
        statsT = small.tile([72, NCH, 6], F32)
        statsB = small.tile([72, NCH, 6], F32)

        def img(buf, h0, nrows=RPC):
            return buf[:, 1 + h0:1 + h0 + nrows, 2:2 + W]

        def tapv(buf, h0, di, dj, nrows=RPC):
            return buf[:, h0 + di:h0 + di + nrows, 1 + dj:1 + dj + W]

        # ======== stage A: per-chunk convs (pairs of chunks) ========
        for qp in range(NCH // 2):
            qa, qb = 2 * qp, 2 * qp + 1
            pks = {}
            # key conv: tap-outer over the chunk pair
            for t in range(9):
                di, dj = taps[t]
                for q in (qa, qb):
                    if t == 0:
                        pks[q] = pk.tile([128, RPC, W], F32, tag="kp", name="pkq")
                    nc.tensor.matmul(
                        pks[q][:], WK2[:, t, :], tapv(X2, q * RPC, di, dj),
                        start=(t == 0), stop=(t == 8),
                    )
            k2s = {}
            for q in (qa, qb):
                k2s[q] = w1p_pool.tile([128, CH], BF16, tag="k2", name="k2q")
                nc.scalar.activation(
                    k2s[q][:].rearrange("p (a b) -> p a b", a=RPC),
                    pks[q][:], AF.Relu)
            # e1: two accumulating 64-contraction matmuls
            p1s = {}
            for q in (qa, qb):
                p1s[q] = pe1.tile([64, CH], F32, tag="e1", name="p1q")
                nc.tensor.matmul(
                    p1s[q][:],
                    W1PX[:], img(X2, q * RPC),
                    start=True, stop=False)
            for q in (qa, qb):
                nc.tensor.matmul(
                    p1s[q][:], W1PK[:], k2s[q][:], start=False, stop=True)
            w1cs = {}
            for q in (qa, qb):
                w1cs[q] = w1p_pool.tile([64, CH], BF16, tag="w1c", name="w1cq")
                nc.scalar.activation(w1cs[q][:], p1s[q][:], AF.Relu)
            # e2 per half (+bias on the ACT copy out of psum)
            for q in (qa, qb):
                qs = slice(q * CH, (q + 1) * CH)
                pt = pe2t.tile([72, CH], F32, tag="e2t")
                nc.tensor.matmul(pt[:], W2P[0:32, :], w1cs[q][0:32, :],
                                 start=True, stop=True)
                nc.scalar.activation(W72T[:, qs], pt[:], AF.Identity,
                                     bias=BE2[:])
                pb = pe2b.tile([72, CH], F32, tag="e2b")
                nc.tensor.matmul(pb[:], W2P[32:64, :], w1cs[q][32:64, :],
                                 start=True, stop=True)
                nc.scalar.activation(W72B[:, qs], pb[:], AF.Identity,
                                     bias=BE2[:])
            # c1 (reuses the key psum ring)
            for q in (qa, qb):
                pc = pk.tile([128, RPC, W], F32, tag="kp")
                nc.tensor.matmul(pc[:], WC2[:], img(X2, q * RPC),
                                 start=True, stop=True)
                nc.vector.tensor_copy(img(XV2, q * RPC), pc[:])
            # GN statistics (bn_stats free dim is capped at 512)
            for q in (qa, qb):
                qs = slice(q * CH, (q + 1) * CH)
                nc.vector.bn_stats(out=statsT[:, q, :], in_=W72T[:, qs])
                nc.vector.bn_stats(out=statsB[:, q, :], in_=W72B[:, qs])
            if qp == 0:
                # bottom half's upper halo = top half's chunk-0 output... no:
                # XV2[64:128, 0] (bottom halo) = xv image row 63 -> top half
                # local row 63 = chunk 15 (NOT ready yet).
                # XV2[0:64, 65] (top halo) = xv image row 64 -> bottom half
                # local row 0 = chunk 0 (ready now).
                nc.sync.dma_start(XV2[0:64, 65:66, 2:130], XV2[64:128, 1:2, 2:130])
            if qp == NCH // 2 - 1:
                # on the scalar queue: the sync queue may be stalled on wb
                # ring slots whose release needs stage C (which reads this)
                nc.scalar.dma_start(XV2[64:128, 0:1, 2:130], XV2[0:64, 64:65, 2:130])
            if qp % 2 == 1 and qp // 2 < 3:
                # raw kernel broadcasts for the completed big chunk (sync
                # hwdge; GN scale/bias is applied post-broadcast).  Only the
                # first two big chunks are prefetched raw; the last two are
                # broadcast after GN is applied directly on W72.
                bqd = qp // 2
                bs = slice(bqd * BQ, (bqd + 1) * BQ)
                for t in range(9):
                    wb = wbp.tile([128, BQ], BF16, tag="wb", name="wbt")
                    sT = W72T[:][t:72:9, bs].unsqueeze(1).broadcast_to([8, 8, BQ])
                    sB = W72B[:][t:72:9, bs].unsqueeze(1).broadcast_to([8, 8, BQ])
                    nc.sync.dma_start(wb[0:64, :], sT)
                    nc.sync.dma_start(wb[64:128, :], sB)
                    wbs[(bqd, t)] = wb

        # ======== stage B: GroupNorm scale/bias ========
        mvT = small.tile([72, 2], F32)
        nc.vector.bn_aggr(out=mvT[:], in_=statsT[:])
        mvB = small.tile([72, 2], F32)
        nc.vector.bn_aggr(out=mvB[:], in_=statsB[:])
        packT = small.tile([72, 2], BF16)
        packB = small.tile([72, 2], BF16)
        for mv, pack in ((mvT, packT), (mvB, packB)):
            rowq = small.tile([72, 1], F32, tag="rowq")
            nc.vector.tensor_mul(rowq[:], mv[:, 0:1], mv[:, 0:1])
            nc.vector.tensor_add(rowq[:], rowq[:], mv[:, 1:2])
            nc.vector.tensor_copy(pack[:, 0:1], mv[:, 0:1])
            nc.vector.tensor_copy(pack[:, 1:2], rowq[:])
        pg = psm.tile([128, 2], F32, tag="sp")
        nc.tensor.matmul(pg[0:8, :], G18[:], packT[:], start=True, stop=False)
        nc.tensor.matmul(pg[0:8, :], G18[:], packB[:], start=False, stop=True)
        gm = small.tile([8, 2], F32)
        nc.vector.tensor_copy(gm[:], pg[0:8, :])
        msq = small.tile([8, 1], F32)
        nc.vector.tensor_mul(msq[:], gm[:, 0:1], gm[:, 0:1])
        v8 = small.tile([8, 1], F32)
        nc.vector.tensor_tensor(out=v8[:], in0=gm[:, 1:2], in1=msq[:],
                                op=ALU.subtract)
        eps8 = small.tile([8, 1], F32)
        nc.vector.memset(eps8[:], EPS)
        sd8 = small.tile([8, 1], F32)
        nc.scalar.activation(sd8[:], v8[:], AF.Sqrt, bias=eps8[:])
        rstd8 = small.tile([8, 2], F32)
        nc.vector.reciprocal(rstd8[:, 0:1], sd8[:])
        nc.vector.tensor_copy(rstd8[:, 1:2], gm[:, 0:1])
        rstd8b = small.tile([8, 2], BF16)
        nc.vector.tensor_copy(rstd8b[:], rstd8[:])
        # broadcast (rstd, m) to channels (tap-independent), then build the
        # per-(channel, tap) scale/bias tables with wide DVE ops
        p128 = psm.tile([128, 2], F32, tag="sp")
        nc.tensor.matmul(p128[:], B128[:], rstd8b[:], start=True, stop=True)
        r128 = small.tile([128, 2], F32)
        nc.vector.tensor_copy(r128[:], p128[:])
        ABTa = small.tile([128, 9], F32)
        nc.vector.tensor_scalar(out=ABTa[:], in0=GNWI[:], scalar1=r128[:, 0:1],
                                scalar2=None, op0=ALU.mult)
        ABTb = small.tile([128, 9], F32)
        nc.vector.tensor_scalar(out=ABTb[:], in0=ABTa[:], scalar1=r128[:, 1:2],
                                scalar2=None, op0=ALU.mult)
        nc.vector.tensor_tensor(out=ABTb[:], in0=GNBI[:], in1=ABTb[:],
                                op=ALU.subtract)
        # per-row scale/bias for the in-place W72 GN-apply (bq2/3 source)
        p72 = psm.tile([128, 2], F32, tag="sp")
        nc.tensor.matmul(p72[0:72, :], B72[:], rstd8b[:], start=True, stop=True)
        rs72 = small.tile([72, 2], F32)
        nc.vector.tensor_copy(rs72[:], p72[0:72, :])
        a72 = small.tile([72, 1], F32)
        nc.vector.tensor_mul(a72[:], rs72[:, 0:1], GNW[:])
        b72 = small.tile([72, 1], F32)
        nc.vector.tensor_mul(b72[:], rs72[:, 1:2], a72[:])
        nc.vector.tensor_tensor(out=b72[:], in0=GNB[:], in1=b72[:],
                                op=ALU.subtract)

        pe2b.release()
        pe2t.release()
        pe1.release()
        pk.release()
        psm.release()
        pcs = tc.alloc_tile_pool(name="pcs", bufs=8, space="PSUM")

        # ======== stage C: normalize prefetched kernels + local conv ========
        # products on DVE; the tap-sum runs on the (idle) PE as accumulating
        # identity matmuls into PSUM (f32), drained by DVE copies w/ row-sums
        ys16 = small.tile([128, NCH], F32)
        for bq in range(NBQ):
            r0 = bq * RBQ
            banks = {}
            pa = None
            for t in range(9):
                di, dj = taps[t]
                wb = wbs.pop((bq, t))
                if bq == 3:
                    pass  # already normalized at the source
                elif bq == 1 or t in (0, 1, 2, 3, 7):
                    # bq1 stays off gpsimd: the gp queue carries the bq3
                    # broadcast issues which stall on wb ring slots that
                    # bq1's products release
                    nc.scalar.activation(wb[:], wb[:], AF.Identity,
                                         bias=ABTb[:, t:t + 1], scale=ABTa[:, t:t + 1])
                else:
                    nc.gpsimd.tensor_scalar(
                        out=wb[:], in0=wb[:], scalar1=ABTa[:, t:t + 1],
                        scalar2=ABTb[:, t:t + 1], op0=ALU.mult, op1=ALU.add)
                wbv = wb[:].rearrange("p (a b) -> p a b", a=RBQ)
                xs = tapv(XV2, r0, di, dj, nrows=RBQ)
                if t < 6:
                    # PE-summed taps
                    p = prp.tile([128, BQ], BF16, tag="p")
                    pv = p[:].rearrange("p (a b) -> p a b", a=RBQ)
                    nc.vector.tensor_mul(pv, xs, wbv)
                    for c in range(4):
                        if t == 0:
                            banks[c] = pcs.tile([128, CH], F32, tag="cb",
                                                name="cbank")
                        nc.tensor.matmul(
                            banks[c][:], I128[:], p[:, c * CH:(c + 1) * CH],
                            start=(t == 0), stop=(t == 5))
                elif t == 6:
                    pa = prp.tile([128, BQ], BF16, tag="pa", name="pacc", bufs=2)
                    nc.vector.tensor_mul(
                        pa[:].rearrange("p (a b) -> p a b", a=RBQ), xs, wbv)
                else:
                    p = prp.tile([128, BQ], BF16, tag="p")
                    pv = p[:].rearrange("p (a b) -> p a b", a=RBQ)
                    nc.vector.tensor_mul(pv, xs, wbv)
                    nc.vector.tensor_add(pa[:], pa[:], p[:])
            for c in range(4):
                q = bq * 4 + c
                nc.vector.tensor_tensor(
                    out=OUT2[:, q * CH:(q + 1) * CH], in0=banks[c][:],
                    in1=pa[:, c * CH:(c + 1) * CH], op=ALU.add)
                nc.vector.tensor_reduce(
                    ys16[:, q:q + 1], OUT2[:, q * CH:(q + 1) * CH],
                    axis=mybir.AxisListType.X, op=ALU.add)
            if bq == 0:
                # GN-apply in place on the second half of W72, then broadcast
                # the already-normalized kernels for big chunks 2-3 (overlaps
                # with big-chunk-1 compute)
                half = slice(3 * BQ, 4 * BQ)
                nc.scalar.activation(W72T[:, half], W72T[:, half], AF.Identity,
                                     bias=b72[:], scale=a72[:])
                nc.gpsimd.tensor_scalar(out=W72B[:, half], in0=W72B[:, half],
                                        scalar1=a72[:], scalar2=b72[:],
                                        op0=ALU.mult, op1=ALU.add)
                for bqd in (3,):
                    bs2 = slice(bqd * BQ, (bqd + 1) * BQ)
                    for t in range(9):
                        wb = wbp.tile([128, BQ], BF16, tag="wb", name="wbt")
                        sT = W72T[:][t:72:9, bs2].unsqueeze(1).broadcast_to([8, 8, BQ])
                        sB = W72B[:][t:72:9, bs2].unsqueeze(1).broadcast_to([8, 8, BQ])
                        nc.sync.dma_start(wb[0:64, :], sT)
                        nc.sync.dma_start(wb[64:128, :], sB)
                        wbs[(bqd, t)] = wb

        pcs.release()
        psm2 = tc.alloc_tile_pool(name="psm2", bufs=2, space="PSUM")

        # ======== stage D: channel attention + store ========
        ysum = small.tile([128, 1], F32)
        nc.vector.tensor_reduce(ysum[:], ys16[:], axis=mybir.AxisListType.X,
                                op=ALU.add)
        ysb = small.tile([128, 1], BF16)
        nc.vector.tensor_copy(ysb[:], ysum[:])
        pa1 = psm2.tile([128, 2], F32, tag="sp")
        nc.tensor.matmul(pa1[0:4, 0:1], WDU1[:], ysb[:], start=True, stop=True)
        y1 = small.tile([4, 1], BF16)
        nc.scalar.activation(y1[:], pa1[0:4, 0:1], AF.Relu)
        pa2 = psm2.tile([128, 2], F32, tag="sp")
        nc.tensor.matmul(pa2[:, 0:1], WDU2[:], y1[:], start=True, stop=True)
        yatt = small.tile([128, 1], F32)
        nc.scalar.activation(yatt[:], pa2[:, 0:1], AF.Sigmoid)

        for hb in range(NBQ):
            r0 = hb * RBQ
            bs = slice(hb * BQ, (hb + 1) * BQ)
            ot = otp.tile([128, RBQ, W], BF16, tag="ot")
            if hb % 2 == 0:
                nc.scalar.activation(
                    ot[:].rearrange("p a b -> p (a b)"), OUT2[:, bs],
                    AF.Identity, scale=yatt[:])
            else:
                nc.vector.tensor_scalar_mul(
                    ot[:].rearrange("p a b -> p (a b)"), OUT2[:, bs], yatt[:])
            otf = ot[:].bitcast(F32)
            nc.sync.dma_start(out_d[:, r0:r0 + RBQ, :], otf[0:64])
            nc.scalar.dma_start(out_d[:, 64 + r0:64 + r0 + RBQ, :], otf[64:128])
        psm2.release()


def _b128():
    m = np.zeros((8, 128), np.float32)
    for c in range(128):
        m[(c % 64) // 8, c] = 1.0
    return m


def _gn_per_tap(v):
    v = np.asarray(v, np.float32).reshape(72)
    out = np.zeros((128, 9), np.float32)
    for c in range(128):
        g = (c % 64) // 8
        for t in range(9):
            out[c, t] = v[g * 9 + t]
    return out


def prep_weights(w_key, w_e1, w_e2, b_e2, gn_w, gn_b, w_c1, w_du1, w_du2):
    import ml_dtypes

    bf = ml_dtypes.bfloat16
    # key conv: per tap block-diag [9, 128, 128]
    wk2 = np.zeros((128, 9, 128), np.float32)
    for t in range(9):
        di, dj = t // 3, t % 3
        base = np.zeros((64, 64), np.float32)
        for o in range(64):
            g = o // 8
            for j in range(8):
                base[g * 8 + j, o] = w_key[o, j, di, dj]
        wk2[0:64, t, 0:64] = base
        wk2[64:128, t, 64:128] = base

    # e1: split x/k parts with halves stacking
    w1px = np.zeros((64, 32), np.float32)
    w1pk = np.zeros((64, 32), np.float32)
    for r in range(64):
        qx = 2 * r       # x channel r in qk interleave
        qk_ = 2 * r + 1  # k channel r
        if qx < 64:
            w1px[r, 0:16] = w_e1[0:16, qx, 0, 0]
        else:
            w1px[r, 16:32] = w_e1[16:32, qx - 64, 0, 0]
        if qk_ < 64:
            w1pk[r, 0:16] = w_e1[0:16, qk_, 0, 0]
        else:
            w1pk[r, 16:32] = w_e1[16:32, qk_ - 64, 0, 0]
    w1px2 = np.zeros((128, 64), np.float32)
    w1px2[0:64, 0:32] = w1px
    w1px2[64:128, 32:64] = w1px
    w1pk2 = np.zeros((128, 64), np.float32)
    w1pk2[0:64, 0:32] = w1pk
    w1pk2[64:128, 32:64] = w1pk

    # e2 (per half): [32, 72], duplicated to rows 32:64 for the
    # bottom-half matmul (fmap and weights must share a base partition)
    w2p1 = np.zeros((32, 72), np.float32)
    for j in range(32):
        if j < 16:
            w2p1[j, 0:36] = w_e2[0:36, j, 0, 0]
        else:
            w2p1[j, 36:72] = w_e2[36:72, j - 16, 0, 0]
    w2p = np.vstack([w2p1, w2p1])

    # c1 block-diag
    wc1 = np.zeros((64, 64), np.float32)
    for o in range(64):
        if o < 32:
            wc1[0:32, o] = w_c1[o, :, 0, 0]
        else:
            wc1[32:64, o] = w_c1[o, :, 0, 0]
    wc2 = np.zeros((128, 128), np.float32)
    wc2[0:64, 0:64] = wc1
    wc2[64:128, 64:128] = wc1

    g18 = np.zeros((72, 8), np.float32)
    for r in range(72):
        g18[r, r // 9] = 1.0 / 18.0
    b72m = np.zeros((8, 72), np.float32)
    for r in range(72):
        b72m[r // 9, r] = 1.0

    wdu1 = (w_du1[:, :, 0, 0].T / float(NPIX)).astype(np.float32)  # [64, 4]
    wdu12 = np.vstack([wdu1, wdu1])                                # [128, 4]
    wdu2 = w_du2[:, :, 0, 0].T.astype(np.float32)                  # [4, 64]
    wdu22 = np.hstack([wdu2, wdu2])                                # [4, 128]

    return {
        "wk2": wk2.astype(bf),
        "w1px2": w1px2.astype(bf),
        "w1pk2": w1pk2.astype(bf),
        "w2p": w2p.astype(bf),
        "wc2": wc2.astype(bf),
        "b_e2": b_e2.reshape(72, 1).astype(np.float32),
        "gn_w": gn_w.reshape(72, 1).astype(np.float32),
        "gn_b": gn_b.reshape(72, 1).astype(np.float32),
        "g18": g18.astype(bf),
        "b72m": b72m.astype(bf),
        "wdu12": wdu12.astype(bf),
        "wdu22": wdu22.astype(bf),
        "i128": np.eye(128, dtype=np.float32).astype(bf),
        "b128": _b128().astype(bf),
        "gnwi": _gn_per_tap(gn_w),
        "gnbi": _gn_per_tap(gn_b),
    }


def emulate(x, wm):
    """Numpy emulation of the on-core dataflow (f32; validates index maps)."""
    def half_stack(a):  # [64, 128, 128] -> [128, 64, 128]
        return np.concatenate([a[:, 0:64], a[:, 64:128]], axis=0)

    xs = half_stack(x)  # [128, 64, 128]
    # padded X2
    X2 = np.zeros((128, PH2, PW), np.float32)
    X2[:, 1:65, 2:130] = xs
    X2[0:64, 65, 2:130] = x[:, 64]
    X2[64:128, 0, 2:130] = x[:, 63]

    def tapv(buf, di, dj):  # full-image tap view [128, 64, 128]
        return buf[:, di:di + 64, 1 + dj:1 + dj + 128]

    # key conv
    pk = np.zeros((128, 64, 128), np.float32)
    for t in range(9):
        di, dj = t // 3, t % 3
        pk += np.einsum('io,ihw->ohw', wm["wk2"][:, t, :].astype(np.float32),
                        tapv(X2, di, dj))
    K2 = np.maximum(pk, 0)
    # e1
    p1 = (np.einsum('io,ihw->ohw', wm["w1px2"].astype(np.float32), X2[:, 1:65, 2:130])
          + np.einsum('io,ihw->ohw', wm["w1pk2"].astype(np.float32), K2))
    W1c = np.maximum(p1, 0)  # [64, 64, 128]
    # e2 per half + bias
    w2p = wm["w2p"][0:32].astype(np.float32)
    be2 = wm["b_e2"].astype(np.float32)
    W72T = np.einsum('io,ihw->ohw', w2p, W1c[0:32]) + be2[:, None]
    W72B = np.einsum('io,ihw->ohw', w2p, W1c[32:64]) + be2[:, None]
    # GN over both halves
    cat = np.stack([W72T, W72B], axis=1).reshape(8, 18, 64, 128)
    m = cat.mean(axis=(1, 2, 3), keepdims=True)
    v = cat.var(axis=(1, 2, 3), keepdims=True)
    a = (wm["gn_w"].astype(np.float32).reshape(8, 9, 1, 1, 1)
         / np.sqrt(v[:, None, 0] + EPS)[..., None].transpose(0, 1, 2, 3, 4)[:, :, 0:1])
    # simpler: compute rstd per group then per row
    rstd = 1.0 / np.sqrt(v.reshape(8) + EPS)
    mg = m.reshape(8)
    gw = wm["gn_w"].astype(np.float32).reshape(72)
    gb = wm["gn_b"].astype(np.float32).reshape(72)
    alpha = gw * rstd[np.arange(72) // 9]
    beta = gb - mg[np.arange(72) // 9] * alpha
    W72T = W72T * alpha[:, None, None] + beta[:, None, None]
    W72B = W72B * alpha[:, None, None] + beta[:, None, None]
    # c1
    pc = np.einsum('io,ihw->ohw', wm["wc2"].astype(np.float32), X2[:, 1:65, 2:130])
    XV2 = np.zeros((128, PH2, PW), np.float32)
    XV2[:, 1:65, 2:130] = pc
    XV2[0:64, 65, 2:130] = pc[64:128, 0]
    XV2[64:128, 0, 2:130] = pc[0:64, 63]
    # local conv
    OUT2 = np.zeros((128, 64, 128), np.float32)
    gidx = np.arange(128) // 8 * 9  # base row per channel (mod 72 within half)
    for t in range(9):
        di, dj = t // 3, t % 3
        xsv = tapv(XV2, di, dj)
        wbT = W72T[(np.arange(64) // 8) * 9 + t]
        wbB = W72B[(np.arange(64) // 8) * 9 + t]
        wb = np.concatenate([wbT, wbB], axis=0)
        OUT2 += xsv * wb
    # attention
    ysum = OUT2.sum(axis=(1, 2))
    y = ysum @ (wm["wdu12"].astype(np.float32))  # includes both halves + 1/NPIX
    y = np.maximum(y, 0)
    y = y @ wm["wdu22"].astype(np.float32)
    y = 1.0 / (1.0 + np.exp(-y))
    OUT2 = OUT2 * y[:, None, None]
    out = np.concatenate([OUT2[0:64], OUT2[64:128]], axis=1)
    return out


_PROGRAM_CACHE = {}


def _get_program():
    if "nc" not in _PROGRAM_CACHE:
        _PROGRAM_CACHE["nc"] = build_program()
    return _PROGRAM_CACHE["nc"]


def run_on_cores(inputs, trace=False):
    nc = _get_program()
    x = np.asarray(inputs["x"], np.float32)
    wmaps = prep_weights(
        np.asarray(inputs["w_key"], np.float32),
        np.asarray(inputs["w_e1"], np.float32),
        np.asarray(inputs["w_e2"], np.float32),
        np.asarray(inputs["b_e2"], np.float32),
        np.asarray(inputs["gn_w"], np.float32),
        np.asarray(inputs["gn_b"], np.float32),
        np.asarray(inputs["w_c1"], np.float32),
        np.asarray(inputs["w_du1"], np.float32),
        np.asarray(inputs["w_du2"], np.float32),
    )
    in_maps = []
    for b in range(8):
        m = {"x_shard": np.ascontiguousarray(x[b])}
        m.update(wmaps)
        in_maps.append(m)
    res = run_bass_kernel_spmd(nc, in_maps, core_ids=list(range(8)), trace=trace)
    import ml_dtypes
    outs = []
    for b in range(8):
        raw = np.ascontiguousarray(np.asarray(res.results[b]["out"], np.float32))
        bf = raw.view(ml_dtypes.bfloat16).reshape(C, H, W)
        outs.append(bf.astype(np.float32))
    out = np.stack(outs, axis=0)
    return out, res


def kernel(**inputs) -> np.ndarray:
    out, _ = run_on_cores(inputs, trace=False)
    return out.astype(np.float32)
